# revision 1
# baseline (speedup 1.0000x reference)
"""Complex AttentionPool2d on 8 trn2 NeuronCores, data-parallel over batch.

Contract: kernel(**inputs) takes the FULL inputs from setup_inputs() and
returns the FULL [32, 512] complex64 output.

Math (per batch):
  x = complex(x_real, x_imag).reshape(E, 256); x_cat = [mean(x), x]  # [E, 257]
  x_cat += pos
  q0 = (x_cat[:, 0] @ w_q^T + b_q) / 8          # only query position 0 matters
  k  = x_cat^T @ w_k^T   (k-bias is softmax-invariant -> dropped)
  v  = x_cat^T @ w_v^T   (v-bias folded into final bias on host)
  logits[h, s] = sum_d q0[h*64+d] * k[s, h*64+d]       # complex product
  w = softmax(logits.re) + i*softmax(logits.im)
  attn0 = w @ v                                  # [E]
  y = attn0 @ (w_p @ w_out)^T + b_c              # fused projection, [512]

Sharding: batch 32 -> 4 per core. All big matmuls fp32r (1 cycle/row, N>=256).
"""
import numpy as np

B, E, HW, S = 32, 512, 256, 257
SP = 258            # S padded even for fp32r
NH, HD = 8, 64
OUT = 512
NCORES = 8
BPC = B // NCORES   # batches per core

_cached = {}


def _build():
    import concourse.bacc as bacc
    import concourse.tile as tile
    import concourse.mybir as mybir

    F32 = mybir.dt.float32
    F32R = mybir.dt.float32r
    AX = mybir.AxisListType
    ACTF = mybir.ActivationFunctionType

    nc = bacc.Bacc("TRN2", target_bir_lowering=False, debug=False)

    # ---- DRAM I/O ----
    d_xr = nc.dram_tensor("xr", [BPC, E, HW + 1], F32, kind="ExternalInput")
    d_xi = nc.dram_tensor("xi", [BPC, E, HW + 1], F32, kind="ExternalInput")
    d_posr = nc.dram_tensor("posr", [E, SP], F32, kind="ExternalInput")
    d_posi = nc.dram_tensor("posi", [E, SP], F32, kind="ExternalInput")
    d_wkvr = nc.dram_tensor("wkvr", [E, 2 * E], F32, kind="ExternalInput")
    d_wkvi = nc.dram_tensor("wkvi", [E, 2 * E], F32, kind="ExternalInput")
    d_wqr = nc.dram_tensor("wqr", [E, E], F32, kind="ExternalInput")
    d_wqi = nc.dram_tensor("wqi", [E, E], F32, kind="ExternalInput")
    d_wcr = nc.dram_tensor("wcr", [E, OUT], F32, kind="ExternalInput")
    d_wci = nc.dram_tensor("wci", [E, OUT], F32, kind="ExternalInput")
    d_bqr = nc.dram_tensor("bqr", [128, 4], F32, kind="ExternalInput")
    d_bqi = nc.dram_tensor("bqi", [128, 4], F32, kind="ExternalInput")
    d_bcr = nc.dram_tensor("bcr", [BPC, OUT], F32, kind="ExternalInput")
    d_bci = nc.dram_tensor("bci", [BPC, OUT], F32, kind="ExternalInput")
    d_id = nc.dram_tensor("ident", [128, 128], F32, kind="ExternalInput")
    d_mask = nc.dram_tensor("mask8", [NH, E], F32, kind="ExternalInput")
    d_sel = nc.dram_tensor("sel32", [32, BPC], F32, kind="ExternalInput")
    d_zbd = nc.dram_tensor("zbd", [128, 32], F32, kind="ExternalInput")
    d_yr = nc.dram_tensor("yr", [BPC, OUT], F32, kind="ExternalOutput")
    d_yi = nc.dram_tensor("yi", [BPC, OUT], F32, kind="ExternalOutput")

    with tile.TileContext(nc) as tc:
        with tc.tile_pool(name="consts", bufs=1) as consts, \
             tc.tile_pool(name="vpool", bufs=1) as vpool:
            # ---- persistent weights / constants ----
            wkvr = [consts.tile([128, 2 * E], F32R, name=f"wkvr{e}") for e in range(4)]
            wkvi = [consts.tile([128, 2 * E], F32R, name=f"wkvi{e}") for e in range(4)]
            wkvin = [consts.tile([128, 2 * E], F32R, name=f"wkvin{e}") for e in range(4)]
            wqr = [consts.tile([128, E], F32R, name=f"wqr{e}") for e in range(4)]
            wqi = [consts.tile([128, E], F32R, name=f"wqi{e}") for e in range(4)]
            wcr = [consts.tile([128, OUT], F32R, name=f"wcr{e}") for e in range(4)]
            wci = [consts.tile([128, OUT], F32R, name=f"wci{e}") for e in range(4)]
            posr = [consts.tile([128, SP], F32, name=f"posr{e}") for e in range(4)]
            posi = [consts.tile([128, SP], F32, name=f"posi{e}") for e in range(4)]
            bqr = consts.tile([128, 4], F32)
            bqi = consts.tile([128, 4], F32)
            bqin = consts.tile([128, 4], F32)
            bcr = consts.tile([BPC, OUT], F32)
            bci = consts.tile([BPC, OUT], F32)
            ident = consts.tile([128, 128], F32)
            mask8 = consts.tile([NH, E], F32)
            sel32 = consts.tile([32, BPC], F32)

            # pos first (needed by batch-0 prep); scalar queue so the big
            # sync-queue stream doesn't delay issue
            for e in range(4):
                sl = slice(e * 128, (e + 1) * 128)
                nc.scalar.dma_start(out=posr[e], in_=d_posr.ap()[sl, :])
                nc.scalar.dma_start(out=posi[e], in_=d_posi.ap()[sl, :])

            # v lives until the hv stage; vC and logits live across ktpool exit
            vr = [[vpool.tile([128, OUT], F32R, name=f"vr{b}_{s}")
                   for s in range(2)] for b in range(BPC)]
            vi = [[vpool.tile([128, OUT], F32R, name=f"vi{b}_{s}")
                   for s in range(2)] for b in range(BPC)]
            vCr_sb = vpool.tile([BPC, OUT], F32)
            vCi_sb = vpool.tile([BPC, OUT], F32)
            lg_r = vpool.tile([32, S], F32)
            lg_i = vpool.tile([32, S], F32)

            with tc.tile_pool(name="ktpool", bufs=1) as ktpool:
                kTr = [[ktpool.tile([128, SP], F32R, name=f"kTr{b}_{u}")
                        for u in range(4)] for b in range(BPC)]
                kTi = [[ktpool.tile([128, SP], F32R, name=f"kTi{b}_{u}")
                        for u in range(4)] for b in range(BPC)]
                x0r = [ktpool.tile([128, 4], F32R, name=f"x0r{e}") for e in range(4)]
                x0i = [ktpool.tile([128, 4], F32R, name=f"x0i{e}") for e in range(4)]
                x0in = [ktpool.tile([128, 4], F32R, name=f"x0in{e}") for e in range(4)]
                xlr = [ktpool.tile([128, 4], F32R, name=f"xlr{e}") for e in range(4)]
                xli = [ktpool.tile([128, 4], F32R, name=f"xli{e}") for e in range(4)]

                # ============ PHASE A: x prep + k + v ============
                with tc.tile_pool(name="xpool", bufs=2) as xpool, \
                     tc.tile_pool(name="psA", bufs=2, space="PSUM") as psA:
                    # batch-0 x DMAs land before the weight DMAs in queue order
                    x_pre = {}
                    for e in range(4):
                        sl = slice(e * 128, (e + 1) * 128)
                        xt = xpool.tile([128, SP], F32R, tag=f"xr{e}", name=f"xr_p0_{e}")
                        yt = xpool.tile([128, SP], F32R, tag=f"xi{e}", name=f"xi_p0_{e}")
                        nc.sync.dma_start(out=xt[:, 1:258].bitcast(F32R),
                                          in_=d_xr.ap()[0, sl, :].bitcast(F32R))
                        nc.gpsimd.dma_start(out=yt[:, 1:258].bitcast(F32R),
                                            in_=d_xi.ap()[0, sl, :].bitcast(F32R))
                        for t, pos in ((xt, posr[e]), (yt, posi[e])):
                            with nc.allow_low_precision(reason="f32r holds f32 bits"):
                                nc.vector.reduce_sum(out=t[:, 0:1], in_=t[:, 1:257],
                                                     axis=AX.X)
                            nc.vector.tensor_scalar_mul(t[:, 0:1], t[:, 0:1], 1.0 / HW)
                            nc.vector.tensor_add(t[:], t[:], pos[:])
                        x_pre[e] = (xt, yt)
                    # weights: k-cols first (first k matmul needs them), then v-cols
                    for half in range(2):
                        hs = slice(half * 512, (half + 1) * 512)
                        for e in range(4):
                            sl = slice(e * 128, (e + 1) * 128)
                            nc.sync.dma_start(out=wkvr[e][:, hs].bitcast(F32R),
                                              in_=d_wkvr.ap()[sl, hs].bitcast(F32R))
                            nc.sync.dma_start(out=wkvi[e][:, hs].bitcast(F32R),
                                              in_=d_wkvi.ap()[sl, hs].bitcast(F32R))
                        for e in range(4):
                            with nc.allow_low_precision(reason="f32r holds f32 bits"):
                                nc.vector.tensor_scalar_mul(wkvin[e][:, hs],
                                                            wkvi[e][:, hs], -1.0)
                    # small constants, then late-phase weights
                    nc.sync.dma_start(out=bqr, in_=d_bqr.ap())
                    nc.sync.dma_start(out=bqi, in_=d_bqi.ap())
                    nc.sync.dma_start(out=bcr, in_=d_bcr.ap())
                    nc.sync.dma_start(out=bci, in_=d_bci.ap())
                    nc.sync.dma_start(out=ident, in_=d_id.ap())
                    nc.sync.dma_start(out=mask8, in_=d_mask.ap())
                    nc.sync.dma_start(out=sel32, in_=d_sel.ap())
                    nc.vector.tensor_scalar_mul(bqin, bqi, -1.0)

                    for b in range(BPC):
                        xr_t, xi_t = [], []
                        for e in range(4):
                            sl = slice(e * 128, (e + 1) * 128)
                            if b == 0:
                                xt, yt = x_pre[e]
                            else:
                                xt = xpool.tile([128, SP], F32R, tag=f"xr{e}", name=f"xr_t{b}_{e}")
                                yt = xpool.tile([128, SP], F32R, tag=f"xi{e}", name=f"xi_t{b}_{e}")
                                nc.sync.dma_start(out=xt[:, 1:258].bitcast(F32R),
                                                  in_=d_xr.ap()[b, sl, :].bitcast(F32R))
                                nc.gpsimd.dma_start(out=yt[:, 1:258].bitcast(F32R),
                                                    in_=d_xi.ap()[b, sl, :].bitcast(F32R))
                            xr_t.append(xt)
                            xi_t.append(yt)
                            if b > 0:
                                for t, pos in ((xt, posr[e]), (yt, posi[e])):
                                    with nc.allow_low_precision(reason="f32r holds f32 bits"):
                                        nc.vector.reduce_sum(out=t[:, 0:1],
                                                             in_=t[:, 1:257], axis=AX.X)
                                    nc.vector.tensor_scalar_mul(t[:, 0:1], t[:, 0:1],
                                                                1.0 / HW)
                                    nc.vector.tensor_add(t[:], t[:], pos[:])
                            nc.scalar.copy(x0r[e][:, b:b + 1], xt[:, 0:1])
                            nc.scalar.copy(x0i[e][:, b:b + 1], yt[:, 0:1])
                            nc.scalar.activation(x0in[e][:, b:b + 1], yt[:, 0:1],
                                                 ACTF.Copy, bias=0.0, scale=-1.0)
                            nc.scalar.copy(xlr[e][:, b:b + 1], xt[:, 256:257])
                            nc.scalar.copy(xli[e][:, b:b + 1], yt[:, 256:257])

                        # ---- k^T [f, s]: lhsT = wkv k-cols, rhs = x ----
                        for u in range(4):
                            fs = slice(u * 128, (u + 1) * 128)
                            p1 = psA.tile([128, SP], F32, tag="pk1", name=f"pk1_{b}_{u}")
                            pi = psA.tile([128, SP], F32, tag="pki", name=f"pki_{b}_{u}")
                            for j, (w, x) in enumerate(
                                    [(wkvr[e][:, fs], xr_t[e]) for e in range(4)]
                                    + [(wkvin[e][:, fs], xi_t[e]) for e in range(4)]):
                                nc.tensor.matmul(p1[:], w, x[:], start=(j == 0), stop=(j == 7))
                            for j, (w, x) in enumerate(
                                    [(wkvi[e][:, fs], xr_t[e]) for e in range(4)]
                                    + [(wkvr[e][:, fs], xi_t[e]) for e in range(4)]):
                                nc.tensor.matmul(pi[:], w, x[:], start=(j == 0), stop=(j == 7))
                            nc.vector.tensor_copy(kTr[b][u][:], p1[:])
                            nc.scalar.copy(kTi[b][u][:], pi[:])

                        # ---- v [s, f]: lhsT = x s-block, rhs = wkv v-cols ----
                        for sb in range(2):
                            cs = slice(sb * 128, (sb + 1) * 128)
                            p1 = psA.tile([128, OUT], F32, tag="pv1", name=f"pv1_{b}_{sb}")
                            pi = psA.tile([128, OUT], F32, tag="pvi", name=f"pvi_{b}_{sb}")
                            for j, (x, w) in enumerate(
                                    [(xr_t[e][:, cs], wkvr[e][:, 512:1024]) for e in range(4)]
                                    + [(xi_t[e][:, cs], wkvin[e][:, 512:1024]) for e in range(4)]):
                                nc.tensor.matmul(p1[:], x, w, start=(j == 0), stop=(j == 7))
                            for j, (x, w) in enumerate(
                                    [(xr_t[e][:, cs], wkvi[e][:, 512:1024]) for e in range(4)]
                                    + [(xi_t[e][:, cs], wkvr[e][:, 512:1024]) for e in range(4)]):
                                nc.tensor.matmul(pi[:], x, w, start=(j == 0), stop=(j == 7))
                            nc.vector.tensor_copy(vr[b][sb][:], p1[:])
                            nc.scalar.copy(vi[b][sb][:], pi[:])

                    # late-phase weights: emitted after all x DMAs so they
                    # don't delay the phase-A stream
                    for e in range(4):
                        sl = slice(e * 128, (e + 1) * 128)
                        nc.gpsimd.dma_start(out=wqr[e].bitcast(F32R),
                                            in_=d_wqr.ap()[sl, :].bitcast(F32R))
                        nc.gpsimd.dma_start(out=wqi[e].bitcast(F32R),
                                            in_=d_wqi.ap()[sl, :].bitcast(F32R))
                    for e in range(4):
                        sl = slice(e * 128, (e + 1) * 128)
                        nc.gpsimd.dma_start(out=wcr[e].bitcast(F32R),
                                            in_=d_wcr.ap()[sl, :].bitcast(F32R))
                        nc.gpsimd.dma_start(out=wci[e].bitcast(F32R),
                                            in_=d_wci.ap()[sl, :].bitcast(F32R))

                # ============ PHASE B1: q0 -> bd, vC, logits ============
                with tc.tile_pool(name="miscB1", bufs=1) as mb1:
                    bd_r = mb1.tile([128, 32], F32R)
                    bd_i = mb1.tile([128, 32], F32R)
                    bd_in = mb1.tile([128, 32], F32R)
                    nc.gpsimd.dma_start(out=bd_r[:].bitcast(F32R),
                                        in_=d_zbd.ap()[:].bitcast(F32R))
                    nc.gpsimd.dma_start(out=bd_i[:].bitcast(F32R),
                                        in_=d_zbd.ap()[:].bitcast(F32R))
                    nc.gpsimd.dma_start(out=bd_in[:].bitcast(F32R),
                                        in_=d_zbd.ap()[:].bitcast(F32R))
                    q0r_sb = mb1.tile([BPC, E], F32)
                    q0i_sb = mb1.tile([BPC, E], F32)

                    with tc.tile_pool(name="psB1", bufs=1, space="PSUM") as psB1:
                        # ---- q0 [4b, 512f]: lhsT = x0, rhs = wq^T ----
                        pqr = psB1.tile([BPC, E], F32, tag="pqr")
                        pqi = psB1.tile([BPC, E], F32, tag="pqi")
                        for j, (x, w) in enumerate(
                                [(x0r[e][:], wqr[e][:]) for e in range(4)]
                                + [(x0in[e][:], wqi[e][:]) for e in range(4)]):
                            nc.tensor.matmul(pqr[:], x, w, start=(j == 0), stop=(j == 7))
                        for j, (x, w) in enumerate(
                                [(x0r[e][:], wqi[e][:]) for e in range(4)]
                                + [(x0i[e][:], wqr[e][:]) for e in range(4)]):
                            nc.tensor.matmul(pqi[:], x, w, start=(j == 0), stop=(j == 7))
                        nc.scalar.copy(q0r_sb[:], pqr[:])
                        nc.scalar.copy(q0i_sb[:], pqi[:])

                        # ---- vC: token-256 v row for all batches ----
                        p1 = psB1.tile([BPC, OUT], F32, tag="pc1")
                        pi = psB1.tile([BPC, OUT], F32, tag="pci")
                        for j, (x, w) in enumerate(
                                [(xlr[e][:], wkvr[e][:, 512:1024]) for e in range(4)]
                                + [(xli[e][:], wkvin[e][:, 512:1024]) for e in range(4)]):
                            nc.tensor.matmul(p1[:], x, w, start=(j == 0), stop=(j == 7))
                        for j, (x, w) in enumerate(
                                [(xlr[e][:], wkvi[e][:, 512:1024]) for e in range(4)]
                                + [(xli[e][:], wkvr[e][:, 512:1024]) for e in range(4)]):
                            nc.tensor.matmul(pi[:], x, w, start=(j == 0), stop=(j == 7))
                        nc.scalar.copy(vCr_sb[:], p1[:])
                        nc.scalar.copy(vCi_sb[:], pi[:])

                        # ---- transpose q0 -> bd block-diag [128, 32] ----
                        for u in range(4):
                            fs = slice(u * 128, (u + 1) * 128)
                            ptr = psB1.tile([128, 4], F32, tag="ptq", bufs=2, name=f"ptq{u}")
                            pti = psB1.tile([128, 4], F32, tag="ptj", bufs=2, name=f"ptj{u}")
                            nc.tensor.transpose(ptr[:], q0r_sb[:, fs], ident[0:BPC, 0:BPC])
                            nc.tensor.transpose(pti[:], q0i_sb[:, fs], ident[0:BPC, 0:BPC])
                            for p in range(2):
                                rows = slice(p * 64, (p + 1) * 64)
                                cols = slice(2 * u + p, 32, 8)
                                nc.scalar.activation(bd_r[rows, cols], ptr[rows, :],
                                                     ACTF.Identity,
                                                     bias=bqr[rows, u:u + 1], scale=1.0)
                                nc.scalar.activation(bd_i[rows, cols], pti[rows, :],
                                                     ACTF.Identity,
                                                     bias=bqi[rows, u:u + 1], scale=1.0)
                                nc.scalar.activation(bd_in[rows, cols], pti[rows, :],
                                                     ACTF.Identity,
                                                     bias=bqin[rows, u:u + 1], scale=-1.0)

                    # ---- logits [32, S] (row = b*8 + p*4 + u) ----
                    # two passes (all-real then all-imag) so the real softmax
                    # overlaps the imag logits matmuls on PE
                    with tc.tile_pool(name="psB2", bufs=3, space="PSUM") as psB2:
                        for b in range(BPC):
                            bo_r = mb1.tile([2, 4, SP], F32, tag="bor", bufs=2,
                                            name=f"bo_r{b}")
                            for u in range(4):
                                c0 = b * 8 + 2 * u
                                pr = psB2.tile([2, SP], F32, tag="plr", name=f"plr{b}_{u}")
                                nc.tensor.matmul(pr[:], bd_r[:, c0:c0 + 2], kTr[b][u][:],
                                                 start=True, stop=False)
                                nc.tensor.matmul(pr[:], bd_in[:, c0:c0 + 2], kTi[b][u][:],
                                                 start=False, stop=True)
                                nc.scalar.copy(bo_r[:, u, :], pr[:])
                            nc.sync.dma_start(out=lg_r[b * 8:b * 8 + 8, :],
                                              in_=bo_r[0:2, :, 0:S])
                        for b in range(BPC):
                            bo_i = mb1.tile([2, 4, SP], F32, tag="boi", bufs=2,
                                            name=f"bo_i{b}")
                            for u in range(4):
                                c0 = b * 8 + 2 * u
                                pq = psB2.tile([2, SP], F32, tag="pli", name=f"pli{b}_{u}")
                                nc.tensor.matmul(pq[:], bd_r[:, c0:c0 + 2], kTi[b][u][:],
                                                 start=True, stop=False)
                                nc.tensor.matmul(pq[:], bd_i[:, c0:c0 + 2], kTr[b][u][:],
                                                 start=False, stop=True)
                                nc.vector.tensor_copy(bo_i[:, u, :], pq[:])
                            nc.sync.dma_start(out=lg_i[b * 8:b * 8 + 8, :],
                                              in_=bo_i[0:2, :, 0:S])

            # ============ PHASE B2: softmax, wT, hv, extract, y ============
            with tc.tile_pool(name="miscB2", bufs=1) as mb:
                # vC2[p, b, :]: rows (re, im); vC2s rows (im, re)
                vC2 = mb.tile([2, BPC, OUT], F32R)
                vC2s = mb.tile([2, BPC, OUT], F32R)
                nc.sync.dma_start(out=vC2[0:1, :, :].bitcast(F32R),
                                    in_=vCr_sb[:].bitcast(F32R))
                nc.sync.dma_start(out=vC2[1:2, :, :].bitcast(F32R),
                                    in_=vCi_sb[:].bitcast(F32R))
                nc.sync.dma_start(out=vC2s[0:1, :, :].bitcast(F32R),
                                    in_=vCi_sb[:].bitcast(F32R))
                nc.sync.dma_start(out=vC2s[1:2, :, :].bitcast(F32R),
                                    in_=vCr_sb[:].bitcast(F32R))
                w_ri = mb.tile([32, 2, S], F32)
                w_r = w_ri[:, 0, :]
                w_i = w_ri[:, 1, :]
                for lg, w in ((lg_r, w_r), (lg_i, w_i)):
                    # logits are O(+-8): exp is safe in f32 without max-shift,
                    # and skipping it shortens the serial chain by two hops
                    sm = mb.tile([32, 1], F32, tag="ssm", name=f"sm_{w.name}")
                    rs = mb.tile([32, 1], F32, tag="srs", name=f"rs_{w.name}")
                    nc.scalar.activation(w, lg[:], ACTF.Exp,
                                         bias=0.0, scale=1.0, accum_out=sm[:])
                    nc.vector.reciprocal(rs[:], sm[:])
                    nc.vector.tensor_scalar_mul(w, w, rs[:])

                # ---- transpose w -> wT [S-part, 32] + stacked row-256 tiles ----
                wTr = [mb.tile([128, 32], F32R, name=f"wTr{a}") for a in range(2)]
                wTi = [mb.tile([128, 32], F32R, name=f"wTi{a}") for a in range(2)]
                wTin = [mb.tile([128, 32], F32R, name=f"wTin{a}") for a in range(2)]
                wtc_a = mb.tile([2, 32], F32R)   # rows: wTr_c, -wTi_c
                wtc_b = mb.tile([2, 32], F32R)   # rows: wTr_c, wTi_c
                with tc.tile_pool(name="psB3", bufs=2, space="PSUM") as psB3:
                    for a in range(2):
                        cs = slice(a * 128, (a + 1) * 128)
                        ptr = psB3.tile([128, 32], F32, tag="ptr", name=f"ptr{a}")
                        pti = psB3.tile([128, 32], F32, tag="pti", name=f"pti{a}")
                        nc.tensor.transpose(ptr[:], w_ri[:, 0, cs], ident[0:32, 0:32])
                        nc.tensor.transpose(pti[:], w_ri[:, 1, cs], ident[0:32, 0:32])
                        nc.scalar.copy(wTr[a][:], ptr[:])
                        nc.scalar.copy(wTi[a][:], pti[:])
                        nc.scalar.activation(wTin[a][:], pti[:], ACTF.Copy,
                                             bias=0.0, scale=-1.0)
                    # row-256 of both parts in one [32, 2] -> [2, 32] transpose
                    ptc = psB3.tile([2, 32], F32, tag="ptc")
                    nc.tensor.transpose(ptc[:], w_ri[:, :, 256], ident[0:32, 0:32])
                    wtc_neg = mb.tile([2, 32], F32R)
                    nc.scalar.copy(wtc_b[:], ptc[:])
                    nc.scalar.activation(wtc_neg[:], ptc[:], ACTF.Copy,
                                         bias=0.0, scale=-1.0)
                    # wtc_a rows (re, -im): row copies via DMA (no partition-
                    # alignment restriction there)
                    nc.sync.dma_start(out=wtc_a[0:1, :].bitcast(F32R),
                                      in_=wtc_b[0:1, :].bitcast(F32R))
                    nc.sync.dma_start(out=wtc_a[1:2, :].bitcast(F32R),
                                      in_=wtc_neg[1:2, :].bitcast(F32R))

                # ---- hv: per batch [8, 512]; assemble hvm_all [32, 512] ----
                hvm_r = [mb.tile([NH, OUT], F32, name=f"hvm_r{b}") for b in range(BPC)]
                hvm_i = [mb.tile([NH, OUT], F32, name=f"hvm_i{b}") for b in range(BPC)]
                hvm_all_r = mb.tile([32, OUT], F32)
                hvm_all_i = mb.tile([32, OUT], F32)
                with tc.tile_pool(name="psB4", bufs=2, space="PSUM") as psB4:
                    for b in range(BPC):
                        cols = slice(b * 8, b * 8 + 8)
                        ph_r = psB4.tile([NH, OUT], F32, tag="phr", name=f"phr{b}")
                        ph_i = psB4.tile([NH, OUT], F32, tag="phi", name=f"phi{b}")
                        mm = nc.tensor.matmul
                        mm(ph_r[:], wTr[0][:, cols], vr[b][0][:], start=True, stop=False)
                        mm(ph_r[:], wTr[1][:, cols], vr[b][1][:], start=False, stop=False)
                        mm(ph_r[:], wTin[0][:, cols], vi[b][0][:], start=False, stop=False)
                        mm(ph_r[:], wTin[1][:, cols], vi[b][1][:], start=False, stop=False)
                        mm(ph_r[:], wtc_a[:, cols], vC2[:, b, :], start=False, stop=True)
                        mm(ph_i[:], wTi[0][:, cols], vr[b][0][:], start=True, stop=False)
                        mm(ph_i[:], wTi[1][:, cols], vr[b][1][:], start=False, stop=False)
                        mm(ph_i[:], wTr[0][:, cols], vi[b][0][:], start=False, stop=False)
                        mm(ph_i[:], wTr[1][:, cols], vi[b][1][:], start=False, stop=False)
                        mm(ph_i[:], wtc_b[:, cols], vC2s[:, b, :], start=False, stop=True)
                        nc.vector.tensor_mul(hvm_r[b][:], ph_r[:], mask8[:])
                        nc.vector.tensor_mul(hvm_i[b][:], ph_i[:], mask8[:])
                        nc.sync.dma_start(out=hvm_all_r[b * 8:b * 8 + 8, :],
                                          in_=hvm_r[b][:])
                        nc.sync.dma_start(out=hvm_all_i[b * 8:b * 8 + 8, :],
                                          in_=hvm_i[b][:])

                # ---- extract attn0^T [128, 4] per f-tile via selection matmul ----
                att_r = [mb.tile([128, 4], F32R, name=f"att_r{u}") for u in range(4)]
                att_i = [mb.tile([128, 4], F32R, name=f"att_i{u}") for u in range(4)]
                att_in = [mb.tile([128, 4], F32R, name=f"att_in{u}") for u in range(4)]
                with tc.tile_pool(name="psB5", bufs=2, space="PSUM") as psB5:
                    for u in range(4):
                        fs = slice(u * 128, (u + 1) * 128)
                        par = psB5.tile([128, 4], F32, tag="par", name=f"par{u}")
                        pai = psB5.tile([128, 4], F32, tag="pai", name=f"pai{u}")
                        nc.tensor.matmul(par[:], hvm_all_r[:, fs], sel32[:],
                                         start=True, stop=True)
                        nc.tensor.matmul(pai[:], hvm_all_i[:, fs], sel32[:],
                                         start=True, stop=True)
                        nc.scalar.copy(att_r[u][:], par[:])
                        nc.scalar.copy(att_i[u][:], pai[:])
                        nc.scalar.activation(att_in[u][:], pai[:], ACTF.Copy,
                                             bias=0.0, scale=-1.0)

                # ---- y = attn0 @ Wc^T + b_c ----
                yr_sb = mb.tile([BPC, OUT], F32)
                yi_sb = mb.tile([BPC, OUT], F32)
                with tc.tile_pool(name="psB6", bufs=1, space="PSUM") as psB6:
                    py_r = psB6.tile([BPC, OUT], F32, tag="pyr")
                    py_i = psB6.tile([BPC, OUT], F32, tag="pyi")
                    for j, u in enumerate(range(4)):
                        nc.tensor.matmul(py_r[:], att_r[u][:], wcr[u][:],
                                         start=(j == 0), stop=False)
                        nc.tensor.matmul(py_r[:], att_in[u][:], wci[u][:],
                                         start=False, stop=(j == 3))
                        nc.tensor.matmul(py_i[:], att_r[u][:], wci[u][:],
                                         start=(j == 0), stop=False)
                        nc.tensor.matmul(py_i[:], att_i[u][:], wcr[u][:],
                                         start=False, stop=(j == 3))
                    nc.vector.tensor_add(yr_sb[:], py_r[:], bcr[:])
                    nc.vector.tensor_add(yi_sb[:], py_i[:], bci[:])
                    nc.sync.dma_start(out=d_yr.ap(), in_=yr_sb[:])
                    nc.sync.dma_start(out=d_yi.ap(), in_=yi_sb[:])

    nc.compile()
    return nc


def _host_prep(inputs):
    """Build per-core in_maps from the full inputs."""
    f32 = np.float32
    xr = np.ascontiguousarray(inputs["x_real"], dtype=f32).reshape(B, E, HW)
    xi = np.ascontiguousarray(inputs["x_imag"], dtype=f32).reshape(B, E, HW)
    pos_r = np.asarray(inputs["pos_r"], dtype=f32)
    pos_i = np.asarray(inputs["pos_i"], dtype=f32)
    w_in_r = np.asarray(inputs["w_in_r"], dtype=f32)
    w_in_i = np.asarray(inputs["w_in_i"], dtype=f32)
    b_in_r = np.asarray(inputs["b_in_r"], dtype=f32)
    b_in_i = np.asarray(inputs["b_in_i"], dtype=f32)
    w_out = np.asarray(inputs["w_out_r"], dtype=f32) + 1j * np.asarray(inputs["w_out_i"], dtype=f32)
    b_out = np.asarray(inputs["b_out_r"], dtype=f32) + 1j * np.asarray(inputs["b_out_i"], dtype=f32)
    w_p = np.asarray(inputs["w_p_r"], dtype=f32) + 1j * np.asarray(inputs["w_p_i"], dtype=f32)
    b_p = np.asarray(inputs["b_p_r"], dtype=f32) + 1j * np.asarray(inputs["b_p_i"], dtype=f32)

    posr = np.zeros((E, SP), f32)
    posi = np.zeros((E, SP), f32)
    posr[:, :S] = pos_r
    posi[:, :S] = pos_i

    wkvr = np.ascontiguousarray(w_in_r[E:3 * E].T)          # [E, 2E]
    wkvi = np.ascontiguousarray(w_in_i[E:3 * E].T)
    qs = f32(1.0 / np.sqrt(HD))
    wqr = np.ascontiguousarray(w_in_r[:E].T * qs)           # [E, E]
    wqi = np.ascontiguousarray(w_in_i[:E].T * qs)
    bq_r = (b_in_r[:E] * qs).reshape(4, 128).T.copy()       # [128, 4]
    bq_i = (b_in_i[:E] * qs).reshape(4, 128).T.copy()

    wc = w_p @ w_out                                        # [OUT, E] complex
    wcr = np.ascontiguousarray(wc.real.T.astype(f32))       # [E, OUT]
    wci = np.ascontiguousarray(wc.imag.T.astype(f32))

    b_v = b_in_r[2 * E:] + 1j * b_in_i[2 * E:]
    b_c = (1 + 1j) * (b_v @ wc.T) + b_out @ w_p.T + b_p     # [OUT] complex
    bcr = np.broadcast_to(b_c.real.astype(f32), (BPC, OUT)).copy()
    bci = np.broadcast_to(b_c.imag.astype(f32), (BPC, OUT)).copy()

    ident = np.eye(128, dtype=f32)
    # hv lhsT column c corresponds to head sigma(c) = [0,2,4,6,1,3,5,7][c]
    # (logits rows are stored p-major: row = b*8 + p*4 + u, head = 2u+p)
    sigma = [0, 2, 4, 6, 1, 3, 5, 7]
    mask8 = np.zeros((NH, E), f32)
    for c in range(NH):
        h = sigma[c]
        mask8[c, h * HD:(h + 1) * HD] = 1.0
    sel32 = np.zeros((32, BPC), f32)
    for b in range(BPC):
        sel32[b * 8:(b + 1) * 8, b] = 1.0

    shared = dict(posr=posr, posi=posi, wkvr=wkvr, wkvi=wkvi, wqr=wqr, wqi=wqi,
                  wcr=wcr, wci=wci, bqr=bq_r, bqi=bq_i, bcr=bcr, bci=bci,
                  ident=ident, mask8=mask8, sel32=sel32,
                  zbd=np.zeros((128, 32), f32))
    xrp = np.zeros((B, E, HW + 1), f32)
    xip = np.zeros((B, E, HW + 1), f32)
    xrp[:, :, :HW] = xr
    xip[:, :, :HW] = xi
    in_maps = []
    for c in range(NCORES):
        m = dict(shared)
        m["xr"] = np.ascontiguousarray(xrp[c * BPC:(c + 1) * BPC])
        m["xi"] = np.ascontiguousarray(xip[c * BPC:(c + 1) * BPC])
        in_maps.append(m)
    return in_maps


def _run(inputs, trace=False, **kw):
    from concourse.bass_utils import run_bass_kernel_spmd
    if "nc" not in _cached:
        _cached["nc"] = _build()
    nc = _cached["nc"]
    in_maps = _host_prep(inputs)
    res = run_bass_kernel_spmd(nc, in_maps, core_ids=list(range(NCORES)),
                               trace=trace, **kw)
    out = np.empty((B, OUT), np.complex64)
    for c in range(NCORES):
        out[c * BPC:(c + 1) * BPC] = (res.results[c]["yr"]
                                      + 1j * res.results[c]["yi"])
    return out, res


def kernel(**inputs) -> np.ndarray:
    out, _ = _run(inputs)
    return out



# revision 18
# speedup vs baseline: 1.1950x; 1.1950x over previous
"""Complex AttentionPool2d on 8 trn2 NeuronCores, data-parallel over batch.

Contract: kernel(**inputs) takes the FULL inputs from setup_inputs() and
returns the FULL [32, 512] complex64 output.

V2: all matmuls bf16 (fp32 PSUM accum); k^T eliminated algebraically.
Math (per batch):
  x = bf16(complex(x_real, x_imag)).reshape(E, 256)
  x_cat = [mean(x), x] + pos                       # [E, 257]
  q0 = x_cat[:, 0] @ wq^T + bq                     # only query pos 0 matters
  qk[h, e] = sum_d q0[h*64+d] wk[h*64+d, e]        # fold q into k-proj
  logits[h, s] = sum_e qk[h, e] x_cat[e, s]        # == q0 . k[s]
  w = softmax(logits.re) + i*softmax(logits.im)
  v = x_cat^T @ wv^T                               # [257, 512]
  attn0 = (w @ v) per-head masked; y = attn0 @ (w_p @ w_out)^T + b_c

Sharding: batch 32 -> 4 per core.
"""
import numpy as np

B, E, HW, S = 32, 512, 256, 257
SP = 258            # S padded even
NH, HD = 8, 64
OUT = 512
NCORES = 8
BPC = B // NCORES   # batches per core

_cached = {}


def _build():
    import concourse.bacc as bacc
    import concourse.tile as tile
    import concourse.mybir as mybir

    F32 = mybir.dt.float32
    BF16 = mybir.dt.bfloat16
    AX = mybir.AxisListType
    ACTF = mybir.ActivationFunctionType

    nc = bacc.Bacc("TRN2", target_bir_lowering=False, debug=False)

    # ---- DRAM I/O ----
    # x layout: [E, BPC, SP] so one DMA per e-tile covers all 4 batches;
    # col 0 reserved for the mean token, col 257 zero pad
    d_xr = nc.dram_tensor("xr", [E, BPC, SP], BF16, kind="ExternalInput")
    d_xi = nc.dram_tensor("xi", [E, BPC, SP], BF16, kind="ExternalInput")
    d_posb_r = nc.dram_tensor("posbr", [E, SP], BF16, kind="ExternalInput")
    d_posb_i = nc.dram_tensor("posbi", [E, SP], BF16, kind="ExternalInput")
    d_wqr = nc.dram_tensor("wqr", [E, E], BF16, kind="ExternalInput")
    d_wqi = nc.dram_tensor("wqi", [E, E], BF16, kind="ExternalInput")
    d_wkr = nc.dram_tensor("wkr", [E, E], BF16, kind="ExternalInput")
    d_wki = nc.dram_tensor("wki", [E, E], BF16, kind="ExternalInput")
    d_wvr = nc.dram_tensor("wvr", [E, OUT], BF16, kind="ExternalInput")
    d_wvi = nc.dram_tensor("wvi", [E, OUT], BF16, kind="ExternalInput")
    d_wcr = nc.dram_tensor("wcr", [E, OUT], BF16, kind="ExternalInput")
    d_wci = nc.dram_tensor("wci", [E, OUT], BF16, kind="ExternalInput")
    d_bqr = nc.dram_tensor("bqr", [128, 4], F32, kind="ExternalInput")
    d_bqi = nc.dram_tensor("bqi", [128, 4], F32, kind="ExternalInput")
    d_bcr = nc.dram_tensor("bcr", [BPC, OUT], F32, kind="ExternalInput")
    d_bci = nc.dram_tensor("bci", [BPC, OUT], F32, kind="ExternalInput")
    d_id = nc.dram_tensor("ident", [128, 128], F32, kind="ExternalInput")
    d_mask = nc.dram_tensor("mask8", [NH, OUT], F32, kind="ExternalInput")
    d_sel = nc.dram_tensor("sel32", [32, BPC], BF16, kind="ExternalInput")
    d_zbd = nc.dram_tensor("zbd", [128, 32], BF16, kind="ExternalInput")
    d_yr = nc.dram_tensor("yr", [BPC, OUT], F32, kind="ExternalOutput")
    d_yi = nc.dram_tensor("yi", [BPC, OUT], F32, kind="ExternalOutput")

    with tile.TileContext(nc) as tc:
        with tc.tile_pool(name="consts", bufs=1) as consts, \
             tc.tile_pool(name="vpool", bufs=1) as vpool:
            # ---- persistent weights / constants (bf16) ----
            wvr = [consts.tile([128, OUT], BF16, name=f"wvr{e}") for e in range(4)]
            wvi = [consts.tile([128, OUT], BF16, name=f"wvi{e}") for e in range(4)]
            wvin = [consts.tile([128, OUT], BF16, name=f"wvin{e}") for e in range(4)]
            wqr = [consts.tile([128, E], BF16, name=f"wqr{e}") for e in range(4)]
            wqi = [consts.tile([128, E], BF16, name=f"wqi{e}") for e in range(4)]
            wkr = [consts.tile([128, E], BF16, name=f"wkr{e}") for e in range(4)]
            wki = [consts.tile([128, E], BF16, name=f"wki{e}") for e in range(4)]
            wcr = [consts.tile([128, OUT], BF16, name=f"wcr{e}") for e in range(4)]
            wci = [consts.tile([128, OUT], BF16, name=f"wci{e}") for e in range(4)]
            posb_r = [consts.tile([128, SP], BF16, name=f"posbr{e}") for e in range(4)]
            posb_i = [consts.tile([128, SP], BF16, name=f"posbi{e}") for e in range(4)]
            bqr = consts.tile([128, 4], F32)
            bqi = consts.tile([128, 4], F32)
            bqin = consts.tile([128, 4], F32)
            bcr = consts.tile([BPC, OUT], F32)
            bci = consts.tile([BPC, OUT], F32)
            ident = consts.tile([128, 128], F32)
            mask8 = consts.tile([NH, OUT], F32)
            sel32 = consts.tile([32, BPC], BF16)

            # x tiles: [128e, BPC, SP]
            xbr = [vpool.tile([128, BPC, SP], BF16, name=f"xbr{e}") for e in range(4)]
            xbi = [vpool.tile([128, BPC, SP], BF16, name=f"xbi{e}") for e in range(4)]
            x0in = [vpool.tile([128, BPC], BF16, name=f"x0in{e}") for e in range(4)]
            scr_r = [vpool.tile([128, BPC], F32, name=f"scr_r{e}") for e in range(4)]
            scr_i = [vpool.tile([128, BPC], F32, name=f"scr_i{e}") for e in range(4)]
            # v tiles live until hv
            vr = [[vpool.tile([128, OUT], BF16, name=f"vr{b}_{s}")
                   for s in range(2)] for b in range(BPC)]
            vi = [[vpool.tile([128, OUT], BF16, name=f"vi{b}_{s}")
                   for s in range(2)] for b in range(BPC)]
            vCr_sb = vpool.tile([BPC, OUT], BF16)
            vCi_sb = vpool.tile([BPC, OUT], BF16)
            # bd: per-u zero-padded block-diag q0 [128, 32] (cols b*8+2u+p)
            bd_r = [vpool.tile([128, 32], BF16, name=f"bd_r{u}") for u in range(4)]
            bd_i = [vpool.tile([128, 32], BF16, name=f"bd_i{u}") for u in range(4)]
            bd_in = [vpool.tile([128, 32], BF16, name=f"bd_in{u}") for u in range(4)]
            q0r_sb = vpool.tile([BPC, E], F32)
            q0i_sb = vpool.tile([BPC, E], F32)
            qk_sb_r = vpool.tile([32, E], F32)
            qk_sb_i = vpool.tile([32, E], F32)
            qkT_r = [vpool.tile([128, 32], BF16, name=f"qkTr{e}") for e in range(4)]
            qkT_i = [vpool.tile([128, 32], BF16, name=f"qkTi{e}") for e in range(4)]
            qkT_in = [vpool.tile([128, 32], BF16, name=f"qkTin{e}") for e in range(4)]

            # ---- DMA issue order matters per queue ----
            # sync queue: small consts then x real
            nc.sync.dma_start(out=ident, in_=d_id.ap())
            nc.sync.dma_start(out=sel32, in_=d_sel.ap())
            nc.sync.dma_start(out=mask8, in_=d_mask.ap())
            nc.sync.dma_start(out=bqr, in_=d_bqr.ap())
            nc.sync.dma_start(out=bqi, in_=d_bqi.ap())
            nc.sync.dma_start(out=bcr, in_=d_bcr.ap())
            nc.sync.dma_start(out=bci, in_=d_bci.ap())
            for u in range(4):
                nc.sync.dma_start(out=bd_r[u], in_=d_zbd.ap())
                nc.sync.dma_start(out=bd_i[u], in_=d_zbd.ap())
                nc.sync.dma_start(out=bd_in[u], in_=d_zbd.ap())
            for e in range(4):
                sl = slice(e * 128, (e + 1) * 128)
                nc.sync.dma_start(out=xbr[e][:, :, 1:SP], in_=d_xr.ap()[sl, :, 1:SP])
            # gpsimd queue: x imag then wc (needed last)
            for e in range(4):
                sl = slice(e * 128, (e + 1) * 128)
                nc.gpsimd.dma_start(out=xbi[e][:, :, 1:SP], in_=d_xi.ap()[sl, :, 1:SP])
            for e in range(4):
                sl = slice(e * 128, (e + 1) * 128)
                nc.gpsimd.dma_start(out=wcr[e], in_=d_wcr.ap()[sl, :])
                nc.gpsimd.dma_start(out=wci[e], in_=d_wci.ap()[sl, :])
            # scalar queue: pos (prep needs it), wv (v matmuls start earliest),
            # then wq, wk
            for e in range(4):
                sl = slice(e * 128, (e + 1) * 128)
                nc.scalar.dma_start(out=posb_r[e], in_=d_posb_r.ap()[sl, :])
                nc.scalar.dma_start(out=posb_i[e], in_=d_posb_i.ap()[sl, :])
            for e in range(4):
                sl = slice(e * 128, (e + 1) * 128)
                nc.scalar.dma_start(out=wvr[e], in_=d_wvr.ap()[sl, :])
                nc.scalar.dma_start(out=wvi[e], in_=d_wvi.ap()[sl, :])
            for e in range(4):
                sl = slice(e * 128, (e + 1) * 128)
                nc.scalar.dma_start(out=wqr[e], in_=d_wqr.ap()[sl, :])
                nc.scalar.dma_start(out=wqi[e], in_=d_wqi.ap()[sl, :])
            for e in range(4):
                sl = slice(e * 128, (e + 1) * 128)
                nc.scalar.dma_start(out=wkr[e], in_=d_wkr.ap()[sl, :])
                nc.scalar.dma_start(out=wki[e], in_=d_wki.ap()[sl, :])

            # negated variants computed on device
            nc.vector.tensor_scalar_mul(bqin, bqi, -1.0)
            for e in range(4):
                nc.gpsimd.tensor_scalar_mul(wvin[e][:], wvi[e][:], -1.0)

            # ---- x prep per e-tile (all 4 batches at once):
            #   mean -> col 0 (scalar), then += pos (broadcast over batch) ----
            for e in range(4):
                nc.vector.tensor_reduce(out=scr_r[e][:], in_=xbr[e][:, :, 1:S],
                                        axis=AX.X, op=mybir.AluOpType.add)
                nc.scalar.activation(xbr[e][:, :, 0:1], scr_r[e][:],
                                     ACTF.Copy, bias=0.0, scale=1.0 / HW)
                pb = posb_r[e][:].unsqueeze(1).broadcast_to([128, BPC, SP])
                nc.gpsimd.tensor_add(xbr[e][:], xbr[e][:], pb)
            for e in range(4):
                nc.vector.tensor_reduce(out=scr_i[e][:], in_=xbi[e][:, :, 1:S],
                                        axis=AX.X, op=mybir.AluOpType.add)
                nc.scalar.activation(xbi[e][:, :, 0:1], scr_i[e][:],
                                     ACTF.Copy, bias=0.0, scale=1.0 / HW)
                pb = posb_i[e][:].unsqueeze(1).broadcast_to([128, BPC, SP])
                nc.gpsimd.tensor_add(xbi[e][:], xbi[e][:], pb)
                nc.scalar.activation(x0in[e][:], xbi[e][:, :, 0], ACTF.Copy,
                                     bias=0.0, scale=-1.0)

            mm = nc.tensor.matmul

            with tc.tile_pool(name="psA", bufs=2, space="PSUM") as psA:
                # v rows s in [sb*128, (sb+1)*128) = x cols (col 0 = mean tok)
                def emit_v(b):
                    for sb in range(2):
                        cs = slice(sb * 128, (sb + 1) * 128)
                        p1 = psA.tile([128, OUT], F32, tag="pv1", name=f"pv1_{b}_{sb}")
                        pi = psA.tile([128, OUT], F32, tag="pvi", name=f"pvi_{b}_{sb}")
                        for j, (x, w) in enumerate(
                                [(xbr[e][:, b, cs], wvr[e]) for e in range(4)]
                                + [(xbi[e][:, b, cs], wvin[e]) for e in range(4)]):
                            mm(p1[:], x, w[:], start=(j == 0), stop=(j == 7))
                        for j, (x, w) in enumerate(
                                [(xbr[e][:, b, cs], wvi[e]) for e in range(4)]
                                + [(xbi[e][:, b, cs], wvr[e]) for e in range(4)]):
                            mm(pi[:], x, w[:], start=(j == 0), stop=(j == 7))
                        nc.vector.tensor_copy(vr[b][sb][:], p1[:])
                        nc.scalar.copy(vi[b][sb][:], pi[:])

                for b in range(3):
                    emit_v(b)

                # ============ q0 -> bd ============
                with tc.tile_pool(name="psB1", bufs=1, space="PSUM") as psB1:
                    pqr = psB1.tile([BPC, E], F32, tag="pqr")
                    pqi = psB1.tile([BPC, E], F32, tag="pqi")
                    for j, (x, w) in enumerate(
                            [(xbr[e][:, :, 0], wqr[e][:]) for e in range(4)]
                            + [(x0in[e][:], wqi[e][:]) for e in range(4)]):
                        mm(pqr[:], x, w, start=(j == 0), stop=(j == 7))
                    for j, (x, w) in enumerate(
                            [(xbr[e][:, :, 0], wqi[e][:]) for e in range(4)]
                            + [(xbi[e][:, :, 0], wqr[e][:]) for e in range(4)]):
                        mm(pqi[:], x, w, start=(j == 0), stop=(j == 7))
                    nc.scalar.copy(q0r_sb[:], pqr[:])
                    nc.scalar.copy(q0i_sb[:], pqi[:])

                    # transpose q0 -> bd block-diag [128, 4u, 8]
                    # bd[p*64+d, u, 2b+p] = q0[b, u*128+p*64+d] + bq bias
                    for u in range(4):
                        fs = slice(u * 128, (u + 1) * 128)
                        ptr = psB1.tile([128, 4], F32, tag="ptq", bufs=1, name=f"ptq{u}")
                        pti = psB1.tile([128, 4], F32, tag="ptj", bufs=1, name=f"ptj{u}")
                        nc.tensor.transpose(ptr[:], q0r_sb[:, fs], ident[0:BPC, 0:BPC])
                        nc.tensor.transpose(pti[:], q0i_sb[:, fs], ident[0:BPC, 0:BPC])
                        for p in range(2):
                            rows = slice(p * 64, (p + 1) * 64)
                            cols = slice(2 * u + p, 32, 8)
                            nc.scalar.activation(bd_r[u][rows, cols], ptr[rows, :],
                                                 ACTF.Identity,
                                                 bias=bqr[rows, u:u + 1], scale=1.0)
                            nc.scalar.activation(bd_i[u][rows, cols], pti[rows, :],
                                                 ACTF.Identity,
                                                 bias=bqi[rows, u:u + 1], scale=1.0)
                            nc.scalar.activation(bd_in[u][rows, cols], pti[rows, :],
                                                 ACTF.Identity,
                                                 bias=bqin[rows, u:u + 1], scale=-1.0)

                # last v batch fills PE while bd copies run
                emit_v(3)

                # ============ qk = bd^T @ wk  [rows b*8 + 2u+p, 512e] ============
                with tc.tile_pool(name="psQK", bufs=1, space="PSUM") as psQK:
                    pkr = psQK.tile([32, E], F32, tag="pkr")
                    pki = psQK.tile([32, E], F32, tag="pki")
                    for j, (bdt, w) in enumerate(
                            [(bd_r[u], wkr[u]) for u in range(4)]
                            + [(bd_in[u], wki[u]) for u in range(4)]):
                        mm(pkr[:], bdt[:], w[:], start=(j == 0), stop=(j == 7))
                    for j, (bdt, w) in enumerate(
                            [(bd_r[u], wki[u]) for u in range(4)]
                            + [(bd_i[u], wkr[u]) for u in range(4)]):
                        mm(pki[:], bdt[:], w[:], start=(j == 0), stop=(j == 7))
                    nc.vector.tensor_copy(qk_sb_r[:], pkr[:])
                    nc.scalar.copy(qk_sb_i[:], pki[:])

                # vC: token-256 v row for all batches (fills PE during qk copies)
                with tc.tile_pool(name="psVC", bufs=1, space="PSUM") as psVC:
                    p1 = psVC.tile([BPC, OUT], F32, tag="pc1")
                    pi = psVC.tile([BPC, OUT], F32, tag="pci")
                    for j, (x, w) in enumerate(
                            [(xbr[e][:, :, 256], wvr[e]) for e in range(4)]
                            + [(xbi[e][:, :, 256], wvin[e]) for e in range(4)]):
                        mm(p1[:], x, w[:], start=(j == 0), stop=(j == 7))
                    for j, (x, w) in enumerate(
                            [(xbr[e][:, :, 256], wvi[e]) for e in range(4)]
                            + [(xbi[e][:, :, 256], wvr[e]) for e in range(4)]):
                        mm(pi[:], x, w[:], start=(j == 0), stop=(j == 7))
                    nc.vector.tensor_copy(vCr_sb[:], p1[:])
                    nc.scalar.copy(vCi_sb[:], pi[:])

                # transpose qk -> qkT [128e, 4u, 8] (+ negated imag)
                with tc.tile_pool(name="psQT", bufs=2, space="PSUM") as psQT:
                    for e in range(4):
                        es = slice(e * 128, (e + 1) * 128)
                        ptr = psQT.tile([128, 32], F32, tag="qtr", name=f"qtr{e}")
                        pti = psQT.tile([128, 32], F32, tag="qti", name=f"qti{e}")
                        nc.tensor.transpose(ptr[:], qk_sb_r[:, es], ident[0:32, 0:32])
                        nc.tensor.transpose(pti[:], qk_sb_i[:, es], ident[0:32, 0:32])
                        nc.scalar.copy(qkT_r[e][:], ptr[:])
                        nc.scalar.copy(qkT_i[e][:], pti[:])
                        nc.vector.tensor_scalar_mul(qkT_in[e][:], pti[:], -1.0)

            # ============ logits [8, SP] per batch (row = 2u+p = h) and
            # softmax straight out of PSUM into per-batch w_b tiles ============
            # two passes (all-real then all-imag) so softmax overlaps PE
            with tc.tile_pool(name="miscB2", bufs=1) as mb:
                w_b = [mb.tile([8, 2, S], F32, name=f"w_b{b}") for b in range(BPC)]
                with tc.tile_pool(name="psB2", bufs=3, space="PSUM") as psB2:
                    def softmax(b, ri, psum):
                        # logits are O(+-8): exp safe in f32 without max-shift
                        sm = mb.tile([8, 1], F32, tag="ssm", name=f"sm{b}_{ri}")
                        rs = mb.tile([8, 1], F32, tag="srs", name=f"rs{b}_{ri}")
                        nc.scalar.activation(w_b[b][:, ri, :], psum[:, 0:S],
                                             ACTF.Exp, bias=0.0, scale=1.0,
                                             accum_out=sm[:])
                        nc.vector.reciprocal(rs[:], sm[:])
                        nc.vector.tensor_scalar_mul(w_b[b][:, ri, :],
                                                    w_b[b][:, ri, :], rs[:])

                    for b in range(BPC):
                        pr = psB2.tile([8, SP], F32, tag="plgr", name=f"plgr{b}")
                        for j, (q, x) in enumerate(
                                [(qkT_r[e][:, b * 8:b * 8 + 8], xbr[e][:, b, :])
                                 for e in range(4)]
                                + [(qkT_in[e][:, b * 8:b * 8 + 8], xbi[e][:, b, :])
                                   for e in range(4)]):
                            mm(pr[:], q, x, start=(j == 0), stop=(j == 7))
                        softmax(b, 0, pr)
                    for b in range(BPC):
                        pq = psB2.tile([8, SP], F32, tag="plgi", name=f"plgi{b}")
                        for j, (q, x) in enumerate(
                                [(qkT_r[e][:, b * 8:b * 8 + 8], xbi[e][:, b, :])
                                 for e in range(4)]
                                + [(qkT_i[e][:, b * 8:b * 8 + 8], xbr[e][:, b, :])
                                   for e in range(4)]):
                            mm(pq[:], q, x, start=(j == 0), stop=(j == 7))
                        softmax(b, 1, pq)

                # vC2[p, b, :]: rows (re, im); vC2s rows (im, re)
                vC2 = mb.tile([2, BPC, OUT], BF16)
                vC2s = mb.tile([2, BPC, OUT], BF16)
                nc.sync.dma_start(out=vC2[0:1, :, :], in_=vCr_sb[:])
                nc.sync.dma_start(out=vC2[1:2, :, :], in_=vCi_sb[:])
                nc.sync.dma_start(out=vC2s[0:1, :, :], in_=vCi_sb[:])
                nc.sync.dma_start(out=vC2s[1:2, :, :], in_=vCr_sb[:])

                # ---- transpose w -> wT [S-part, 32] (bf16); col = b*8 + h ----
                wTr = [mb.tile([128, 32], BF16, name=f"wTr{a}") for a in range(2)]
                wTi = [mb.tile([128, 32], BF16, name=f"wTi{a}") for a in range(2)]
                wTin = [mb.tile([128, 32], BF16, name=f"wTin{a}") for a in range(2)]
                wtc_a = mb.tile([2, 32], BF16)   # rows: wTr_c, -wTi_c
                wtc_b = mb.tile([2, 32], BF16)   # rows: wTr_c, wTi_c
                with tc.tile_pool(name="psB3", bufs=1, space="PSUM") as psB3:
                    pw = [[psB3.tile([128, 32], F32, tag=f"pw{a}{ri}",
                                     name=f"pw{a}{ri}")
                           for ri in range(2)] for a in range(2)]
                    ptc = psB3.tile([2, 32], F32, tag="ptc")
                    for b in range(BPC):
                        ocols = slice(b * 8, b * 8 + 8)
                        for a in range(2):
                            cs = slice(a * 128, (a + 1) * 128)
                            for ri in range(2):
                                nc.tensor.matmul(pw[a][ri][:, ocols],
                                                 w_b[b][:, ri, cs],
                                                 ident[0:8, 0:8],
                                                 is_transpose=True,
                                                 skip_group_check=True)
                        nc.tensor.matmul(ptc[:, ocols], w_b[b][:, :, 256],
                                         ident[0:8, 0:8], is_transpose=True,
                                         skip_group_check=True)
                    for a in range(2):
                        nc.scalar.copy(wTr[a][:], pw[a][0][:])
                        nc.scalar.copy(wTi[a][:], pw[a][1][:])
                        nc.scalar.activation(wTin[a][:], pw[a][1][:], ACTF.Copy,
                                             bias=0.0, scale=-1.0)
                    wtc_neg = mb.tile([2, 32], BF16)
                    nc.scalar.copy(wtc_b[:], ptc[:])
                    nc.scalar.activation(wtc_neg[:], ptc[:], ACTF.Copy,
                                         bias=0.0, scale=-1.0)
                    nc.sync.dma_start(out=wtc_a[0:1, :], in_=wtc_b[0:1, :])
                    nc.sync.dma_start(out=wtc_a[1:2, :], in_=wtc_neg[1:2, :])

                # ---- hv: per batch [8, 512] -> mask -> DMA into hvm_all ----
                hvm_r = [mb.tile([NH, OUT], BF16, name=f"hvm_r{b}") for b in range(BPC)]
                hvm_i = [mb.tile([NH, OUT], BF16, name=f"hvm_i{b}") for b in range(BPC)]
                hvm_all_r = mb.tile([32, OUT], BF16)
                hvm_all_i = mb.tile([32, OUT], BF16)
                with tc.tile_pool(name="psB4", bufs=2, space="PSUM") as psB4:
                    for b in range(BPC):
                        cols = slice(b * 8, b * 8 + 8)
                        ph_r = psB4.tile([NH, OUT], F32, tag="phr", name=f"phr{b}")
                        ph_i = psB4.tile([NH, OUT], F32, tag="phi", name=f"phi{b}")
                        mm(ph_r[:], wTr[0][:, cols], vr[b][0][:], start=True, stop=False)
                        mm(ph_r[:], wTr[1][:, cols], vr[b][1][:], start=False, stop=False)
                        mm(ph_r[:], wTin[0][:, cols], vi[b][0][:], start=False, stop=False)
                        mm(ph_r[:], wTin[1][:, cols], vi[b][1][:], start=False, stop=False)
                        mm(ph_r[:], wtc_a[:, cols], vC2[:, b, :], start=False, stop=True)
                        mm(ph_i[:], wTi[0][:, cols], vr[b][0][:], start=True, stop=False)
                        mm(ph_i[:], wTi[1][:, cols], vr[b][1][:], start=False, stop=False)
                        mm(ph_i[:], wTr[0][:, cols], vi[b][0][:], start=False, stop=False)
                        mm(ph_i[:], wTr[1][:, cols], vi[b][1][:], start=False, stop=False)
                        mm(ph_i[:], wtc_b[:, cols], vC2s[:, b, :], start=False, stop=True)
                        nc.vector.tensor_mul(hvm_r[b][:], ph_r[:], mask8[:])
                        nc.vector.tensor_mul(hvm_i[b][:], ph_i[:], mask8[:])
                        nc.sync.dma_start(out=hvm_all_r[b * 8:b * 8 + 8, :],
                                          in_=hvm_r[b][:])
                        nc.sync.dma_start(out=hvm_all_i[b * 8:b * 8 + 8, :],
                                          in_=hvm_i[b][:])

                # ---- extract attn0^T [128, 4] per f-tile via selection matmul ----
                att_r = [mb.tile([128, 4], BF16, name=f"att_r{u}") for u in range(4)]
                att_i = [mb.tile([128, 4], BF16, name=f"att_i{u}") for u in range(4)]
                att_in = [mb.tile([128, 4], BF16, name=f"att_in{u}") for u in range(4)]
                with tc.tile_pool(name="psB5", bufs=2, space="PSUM") as psB5:
                    for u in range(4):
                        fs = slice(u * 128, (u + 1) * 128)
                        par = psB5.tile([128, 4], F32, tag="par", name=f"par{u}")
                        pai = psB5.tile([128, 4], F32, tag="pai", name=f"pai{u}")
                        mm(par[:], hvm_all_r[:, fs], sel32[:], start=True, stop=True)
                        mm(pai[:], hvm_all_i[:, fs], sel32[:], start=True, stop=True)
                        nc.scalar.copy(att_r[u][:], par[:])
                        nc.scalar.copy(att_i[u][:], pai[:])
                        nc.scalar.activation(att_in[u][:], pai[:], ACTF.Copy,
                                             bias=0.0, scale=-1.0)

                # ---- y = attn0 @ Wc^T + b_c ----
                yr_sb = mb.tile([BPC, OUT], F32)
                yi_sb = mb.tile([BPC, OUT], F32)
                with tc.tile_pool(name="psB6", bufs=1, space="PSUM") as psB6:
                    py_r = psB6.tile([BPC, OUT], F32, tag="pyr")
                    py_i = psB6.tile([BPC, OUT], F32, tag="pyi")
                    for j, u in enumerate(range(4)):
                        mm(py_r[:], att_r[u][:], wcr[u][:], start=(j == 0), stop=False)
                        mm(py_r[:], att_in[u][:], wci[u][:], start=False, stop=(j == 3))
                        mm(py_i[:], att_r[u][:], wci[u][:], start=(j == 0), stop=False)
                        mm(py_i[:], att_i[u][:], wcr[u][:], start=False, stop=(j == 3))
                    nc.vector.tensor_add(yr_sb[:], py_r[:], bcr[:])
                    nc.vector.tensor_add(yi_sb[:], py_i[:], bci[:])
                    nc.sync.dma_start(out=d_yr.ap(), in_=yr_sb[:])
                    nc.sync.dma_start(out=d_yi.ap(), in_=yi_sb[:])

    nc.compile()
    return nc


def _host_prep(inputs):
    """Build per-core in_maps from the full inputs."""
    import ml_dtypes
    f32 = np.float32
    bf16 = ml_dtypes.bfloat16
    xr = np.ascontiguousarray(inputs["x_real"], dtype=f32).reshape(B, E, HW)
    xi = np.ascontiguousarray(inputs["x_imag"], dtype=f32).reshape(B, E, HW)
    pos = np.asarray(inputs["pos_r"], dtype=f32) + 1j * np.asarray(inputs["pos_i"], dtype=f32)
    w_in_r = np.asarray(inputs["w_in_r"], dtype=f32)
    w_in_i = np.asarray(inputs["w_in_i"], dtype=f32)
    b_in_r = np.asarray(inputs["b_in_r"], dtype=f32)
    b_in_i = np.asarray(inputs["b_in_i"], dtype=f32)
    w_out = np.asarray(inputs["w_out_r"], dtype=f32) + 1j * np.asarray(inputs["w_out_i"], dtype=f32)
    b_out = np.asarray(inputs["b_out_r"], dtype=f32) + 1j * np.asarray(inputs["b_out_i"], dtype=f32)
    w_p = np.asarray(inputs["w_p_r"], dtype=f32) + 1j * np.asarray(inputs["w_p_i"], dtype=f32)
    b_p = np.asarray(inputs["b_p_r"], dtype=f32) + 1j * np.asarray(inputs["b_p_i"], dtype=f32)

    w_in = w_in_r + 1j * w_in_i
    wq, wk, wv = w_in[:E], w_in[E:2 * E], w_in[2 * E:]
    qs = f32(1.0 / np.sqrt(HD))

    posb = np.zeros((E, SP), np.complex64)
    posb[:, :S] = pos

    wc = w_p @ w_out                                        # [OUT, E] complex
    bq = qs * (b_in_r[:E] + 1j * b_in_i[:E])                # [E]

    b_v = b_in_r[2 * E:] + 1j * b_in_i[2 * E:]
    b_c = (1 + 1j) * (b_v @ wc.T) + b_out @ w_p.T + b_p     # [OUT] complex

    mask8 = np.zeros((NH, OUT), f32)
    for h in range(NH):
        mask8[h, h * HD:(h + 1) * HD] = 1.0
    sel32 = np.zeros((32, BPC), f32)
    for b in range(BPC):
        sel32[b * 8:(b + 1) * 8, b] = 1.0

    shared = dict(
        posbr=posb.real.astype(bf16), posbi=posb.imag.astype(bf16),
        wqr=np.ascontiguousarray(wq.real.T * qs).astype(bf16),
        wqi=np.ascontiguousarray(wq.imag.T * qs).astype(bf16),
        wkr=np.ascontiguousarray(wk.real).astype(bf16),
        wki=np.ascontiguousarray(wk.imag).astype(bf16),
        wvr=np.ascontiguousarray(wv.real.T).astype(bf16),
        wvi=np.ascontiguousarray(wv.imag.T).astype(bf16),
        wcr=np.ascontiguousarray(wc.real.T.astype(f32)).astype(bf16),
        wci=np.ascontiguousarray(wc.imag.T.astype(f32)).astype(bf16),
        bqr=bq.real.astype(f32).reshape(4, 128).T.copy(),
        bqi=bq.imag.astype(f32).reshape(4, 128).T.copy(),
        bcr=np.broadcast_to(b_c.real.astype(f32), (BPC, OUT)).copy(),
        bci=np.broadcast_to(b_c.imag.astype(f32), (BPC, OUT)).copy(),
        ident=np.eye(128, dtype=f32),
        mask8=mask8,
        sel32=sel32.astype(bf16),
        zbd=np.zeros((128, 32), bf16),
    )
    # x: [E, BPC, SP] per core; col 0 reserved for mean, col 257 zero
    xrp = np.zeros((B, E, SP), f32)
    xip = np.zeros((B, E, SP), f32)
    xrp[:, :, 1:1 + HW] = xr
    xip[:, :, 1:1 + HW] = xi
    in_maps = []
    for c in range(NCORES):
        m = dict(shared)
        m["xr"] = np.ascontiguousarray(
            xrp[c * BPC:(c + 1) * BPC].transpose(1, 0, 2)).astype(bf16)
        m["xi"] = np.ascontiguousarray(
            xip[c * BPC:(c + 1) * BPC].transpose(1, 0, 2)).astype(bf16)
        in_maps.append(m)
    return in_maps


def _run(inputs, trace=False, **kw):
    from concourse.bass_utils import run_bass_kernel_spmd
    if "nc" not in _cached:
        _cached["nc"] = _build()
    nc = _cached["nc"]
    in_maps = _host_prep(inputs)
    res = run_bass_kernel_spmd(nc, in_maps, core_ids=list(range(NCORES)),
                               trace=trace, **kw)
    out = np.empty((B, OUT), np.complex64)
    for c in range(NCORES):
        out[c * BPC:(c + 1) * BPC] = (res.results[c]["yr"]
                                      + 1j * res.results[c]["yi"])
    return out, res


def kernel(**inputs) -> np.ndarray:
    out, _ = _run(inputs)
    return out


# revision 20
# speedup vs baseline: 1.5832x; 1.3248x over previous
"""Complex AttentionPool2d on 8 trn2 NeuronCores, data-parallel over batch.

Contract: kernel(**inputs) takes the FULL inputs from setup_inputs() and
returns the FULL [32, 512] complex64 output.

V2: all matmuls bf16 (fp32 PSUM accum); k^T eliminated algebraically.
Math (per batch):
  x = bf16(complex(x_real, x_imag)).reshape(E, 256)
  x_cat = [mean(x), x] + pos                       # [E, 257]
  q0 = x_cat[:, 0] @ wq^T + bq                     # only query pos 0 matters
  qk[h, e] = sum_d q0[h*64+d] wk[h*64+d, e]        # fold q into k-proj
  logits[h, s] = sum_e qk[h, e] x_cat[e, s]        # == q0 . k[s]
  w = softmax(logits.re) + i*softmax(logits.im)
  v = x_cat^T @ wv^T                               # [257, 512]
  attn0 = (w @ v) per-head masked; y = attn0 @ (w_p @ w_out)^T + b_c

Sharding: batch 32 -> 4 per core.
"""
import numpy as np

B, E, HW, S = 32, 512, 256, 257
SP = 258            # S padded even
NH, HD = 8, 64
OUT = 512
NCORES = 8
BPC = B // NCORES   # batches per core

_cached = {}


def _build():
    import concourse.bacc as bacc
    import concourse.tile as tile
    import concourse.mybir as mybir

    F32 = mybir.dt.float32
    BF16 = mybir.dt.bfloat16
    AX = mybir.AxisListType
    ACTF = mybir.ActivationFunctionType

    nc = bacc.Bacc("TRN2", target_bir_lowering=False, debug=False)

    # ---- DRAM I/O ----
    # x layout: [E, BPC, SP] so one DMA per e-tile covers all 4 batches;
    # col 0 reserved for the mean token, col 257 zero pad
    d_xr = nc.dram_tensor("xr", [E, BPC, SP], BF16, kind="ExternalInput")
    d_xi = nc.dram_tensor("xi", [E, BPC, SP], BF16, kind="ExternalInput")
    d_wqr = nc.dram_tensor("wqr", [E, E], BF16, kind="ExternalInput")
    d_wqi = nc.dram_tensor("wqi", [E, E], BF16, kind="ExternalInput")
    d_wkr = nc.dram_tensor("wkr", [E, E], BF16, kind="ExternalInput")
    d_wki = nc.dram_tensor("wki", [E, E], BF16, kind="ExternalInput")
    d_wvr = nc.dram_tensor("wvr", [E, OUT], BF16, kind="ExternalInput")
    d_wvi = nc.dram_tensor("wvi", [E, OUT], BF16, kind="ExternalInput")
    d_wvin = nc.dram_tensor("wvin", [E, OUT], BF16, kind="ExternalInput")
    d_wcr = nc.dram_tensor("wcr", [E, OUT], BF16, kind="ExternalInput")
    d_wci = nc.dram_tensor("wci", [E, OUT], BF16, kind="ExternalInput")
    d_bqr = nc.dram_tensor("bqr", [128, 4], F32, kind="ExternalInput")
    d_bqi = nc.dram_tensor("bqi", [128, 4], F32, kind="ExternalInput")
    d_bcr = nc.dram_tensor("bcr", [BPC, OUT], F32, kind="ExternalInput")
    d_bci = nc.dram_tensor("bci", [BPC, OUT], F32, kind="ExternalInput")
    d_id = nc.dram_tensor("ident", [128, 128], F32, kind="ExternalInput")
    d_mask = nc.dram_tensor("mask8", [NH, OUT], F32, kind="ExternalInput")
    d_sel = nc.dram_tensor("sel32", [32, BPC], BF16, kind="ExternalInput")
    d_zbd = nc.dram_tensor("zbd", [128, 32], BF16, kind="ExternalInput")
    d_yr = nc.dram_tensor("yr", [BPC, OUT], F32, kind="ExternalOutput")
    d_yi = nc.dram_tensor("yi", [BPC, OUT], F32, kind="ExternalOutput")

    with tile.TileContext(nc) as tc:
        with tc.tile_pool(name="consts", bufs=1) as consts, \
             tc.tile_pool(name="vpool", bufs=1) as vpool:
            # ---- persistent weights / constants (bf16) ----
            wvr = [consts.tile([128, OUT], BF16, name=f"wvr{e}") for e in range(4)]
            wvi = [consts.tile([128, OUT], BF16, name=f"wvi{e}") for e in range(4)]
            wvin = [consts.tile([128, OUT], BF16, name=f"wvin{e}") for e in range(4)]
            wqr = [consts.tile([128, E], BF16, name=f"wqr{e}") for e in range(4)]
            wqi = [consts.tile([128, E], BF16, name=f"wqi{e}") for e in range(4)]
            wkr = [consts.tile([128, E], BF16, name=f"wkr{e}") for e in range(4)]
            wki = [consts.tile([128, E], BF16, name=f"wki{e}") for e in range(4)]
            wcr = [consts.tile([128, OUT], BF16, name=f"wcr{e}") for e in range(4)]
            wci = [consts.tile([128, OUT], BF16, name=f"wci{e}") for e in range(4)]
            bqr = consts.tile([128, 4], F32)
            bqi = consts.tile([128, 4], F32)
            bqin = consts.tile([128, 4], F32)
            bcr = consts.tile([BPC, OUT], F32)
            bci = consts.tile([BPC, OUT], F32)
            ident = consts.tile([128, 128], F32)
            mask8 = consts.tile([NH, OUT], F32)
            sel32 = consts.tile([32, BPC], BF16)

            # x tiles: [128e, BPC, SP]
            xbr = [vpool.tile([128, BPC, SP], BF16, name=f"xbr{e}") for e in range(4)]
            xbi = [vpool.tile([128, BPC, SP], BF16, name=f"xbi{e}") for e in range(4)]
            x0in = [vpool.tile([128, BPC], BF16, name=f"x0in{e}") for e in range(4)]
            # v tiles live until hv
            vr = [[vpool.tile([128, OUT], BF16, name=f"vr{b}_{s}")
                   for s in range(2)] for b in range(BPC)]
            vi = [[vpool.tile([128, OUT], BF16, name=f"vi{b}_{s}")
                   for s in range(2)] for b in range(BPC)]
            vCr_sb = vpool.tile([BPC, OUT], BF16)
            vCi_sb = vpool.tile([BPC, OUT], BF16)
            # bd: per-u zero-padded block-diag q0 [128, 32] (cols b*8+2u+p)
            bd_r = [vpool.tile([128, 32], BF16, name=f"bd_r{u}") for u in range(4)]
            bd_i = [vpool.tile([128, 32], BF16, name=f"bd_i{u}") for u in range(4)]
            bd_in = [vpool.tile([128, 32], BF16, name=f"bd_in{u}") for u in range(4)]
            q0r_sb = vpool.tile([BPC, E], F32)
            q0i_sb = vpool.tile([BPC, E], F32)
            qk_sb_r = vpool.tile([32, E], F32)
            qk_sb_i = vpool.tile([32, E], F32)
            qkT_r = [vpool.tile([128, 32], BF16, name=f"qkTr{e}") for e in range(4)]
            qkT_i = [vpool.tile([128, 32], BF16, name=f"qkTi{e}") for e in range(4)]
            qkT_in = [vpool.tile([128, 32], BF16, name=f"qkTin{e}") for e in range(4)]

            # ---- DMA issue order matters per queue ----
            # sync queue: small consts then x real
            nc.sync.dma_start(out=ident, in_=d_id.ap())
            nc.sync.dma_start(out=sel32, in_=d_sel.ap())
            nc.sync.dma_start(out=mask8, in_=d_mask.ap())
            nc.sync.dma_start(out=bqr, in_=d_bqr.ap())
            nc.sync.dma_start(out=bqi, in_=d_bqi.ap())
            nc.sync.dma_start(out=bcr, in_=d_bcr.ap())
            nc.sync.dma_start(out=bci, in_=d_bci.ap())
            for u in range(4):
                nc.sync.dma_start(out=bd_r[u], in_=d_zbd.ap())
                nc.sync.dma_start(out=bd_i[u], in_=d_zbd.ap())
                nc.sync.dma_start(out=bd_in[u], in_=d_zbd.ap())
            # scalar hw queue: x real (v needs it first), wv, wvin
            for e in range(4):
                sl = slice(e * 128, (e + 1) * 128)
                nc.scalar.dma_start(out=xbr[e][:], in_=d_xr.ap()[sl, :, :])
            for e in range(4):
                sl = slice(e * 128, (e + 1) * 128)
                nc.scalar.dma_start(out=wvr[e], in_=d_wvr.ap()[sl, :])
                nc.scalar.dma_start(out=wvi[e], in_=d_wvi.ap()[sl, :])
                nc.scalar.dma_start(out=wvin[e], in_=d_wvin.ap()[sl, :])
            # gpsimd hw queue: x imag, then wq, wk, wc
            for e in range(4):
                sl = slice(e * 128, (e + 1) * 128)
                nc.gpsimd.dma_start(out=xbi[e][:], in_=d_xi.ap()[sl, :, :])
            for e in range(4):
                sl = slice(e * 128, (e + 1) * 128)
                nc.gpsimd.dma_start(out=wqr[e], in_=d_wqr.ap()[sl, :])
                nc.gpsimd.dma_start(out=wqi[e], in_=d_wqi.ap()[sl, :])
            for e in range(4):
                sl = slice(e * 128, (e + 1) * 128)
                nc.gpsimd.dma_start(out=wkr[e], in_=d_wkr.ap()[sl, :])
                nc.gpsimd.dma_start(out=wki[e], in_=d_wki.ap()[sl, :])
            for e in range(4):
                sl = slice(e * 128, (e + 1) * 128)
                nc.gpsimd.dma_start(out=wcr[e], in_=d_wcr.ap()[sl, :])
                nc.gpsimd.dma_start(out=wci[e], in_=d_wci.ap()[sl, :])

            nc.vector.tensor_scalar_mul(bqin, bqi, -1.0)
            # x arrives fully prepped from host (mean in col 0, pos added);
            # only the negated imag of token 0 is built on device
            for e in range(4):
                nc.scalar.activation(x0in[e][:], xbi[e][:, :, 0], ACTF.Copy,
                                     bias=0.0, scale=-1.0)

            mm = nc.tensor.matmul

            with tc.tile_pool(name="psA", bufs=2, space="PSUM") as psA:
                # v rows s in [sb*128, (sb+1)*128) = x cols (col 0 = mean tok)
                def emit_v(b):
                    for sb in range(2):
                        cs = slice(sb * 128, (sb + 1) * 128)
                        p1 = psA.tile([128, OUT], F32, tag="pv1", name=f"pv1_{b}_{sb}")
                        pi = psA.tile([128, OUT], F32, tag="pvi", name=f"pvi_{b}_{sb}")
                        for j, (x, w) in enumerate(
                                [(xbr[e][:, b, cs], wvr[e]) for e in range(4)]
                                + [(xbi[e][:, b, cs], wvin[e]) for e in range(4)]):
                            mm(p1[:], x, w[:], start=(j == 0), stop=(j == 7))
                        for j, (x, w) in enumerate(
                                [(xbr[e][:, b, cs], wvi[e]) for e in range(4)]
                                + [(xbi[e][:, b, cs], wvr[e]) for e in range(4)]):
                            mm(pi[:], x, w[:], start=(j == 0), stop=(j == 7))
                        nc.vector.tensor_copy(vr[b][sb][:], p1[:])
                        nc.scalar.copy(vi[b][sb][:], pi[:])

                for b in range(3):
                    emit_v(b)

                # ============ q0 -> bd ============
                with tc.tile_pool(name="psB1", bufs=1, space="PSUM") as psB1:
                    pqr = psB1.tile([BPC, E], F32, tag="pqr")
                    pqi = psB1.tile([BPC, E], F32, tag="pqi")
                    for j, (x, w) in enumerate(
                            [(xbr[e][:, :, 0], wqr[e][:]) for e in range(4)]
                            + [(x0in[e][:], wqi[e][:]) for e in range(4)]):
                        mm(pqr[:], x, w, start=(j == 0), stop=(j == 7))
                    for j, (x, w) in enumerate(
                            [(xbr[e][:, :, 0], wqi[e][:]) for e in range(4)]
                            + [(xbi[e][:, :, 0], wqr[e][:]) for e in range(4)]):
                        mm(pqi[:], x, w, start=(j == 0), stop=(j == 7))
                    nc.scalar.copy(q0r_sb[:], pqr[:])
                    nc.scalar.copy(q0i_sb[:], pqi[:])

                    # transpose q0 -> bd block-diag [128, 4u, 8]
                    # bd[p*64+d, u, 2b+p] = q0[b, u*128+p*64+d] + bq bias
                    for u in range(4):
                        fs = slice(u * 128, (u + 1) * 128)
                        ptr = psB1.tile([128, 4], F32, tag="ptq", bufs=1, name=f"ptq{u}")
                        pti = psB1.tile([128, 4], F32, tag="ptj", bufs=1, name=f"ptj{u}")
                        nc.tensor.transpose(ptr[:], q0r_sb[:, fs], ident[0:BPC, 0:BPC])
                        nc.tensor.transpose(pti[:], q0i_sb[:, fs], ident[0:BPC, 0:BPC])
                        for p in range(2):
                            rows = slice(p * 64, (p + 1) * 64)
                            cols = slice(2 * u + p, 32, 8)
                            nc.scalar.activation(bd_r[u][rows, cols], ptr[rows, :],
                                                 ACTF.Identity,
                                                 bias=bqr[rows, u:u + 1], scale=1.0)
                            nc.scalar.activation(bd_i[u][rows, cols], pti[rows, :],
                                                 ACTF.Identity,
                                                 bias=bqi[rows, u:u + 1], scale=1.0)
                            nc.scalar.activation(bd_in[u][rows, cols], pti[rows, :],
                                                 ACTF.Identity,
                                                 bias=bqin[rows, u:u + 1], scale=-1.0)

                # last v batch fills PE while bd copies run
                emit_v(3)

                # ============ qk = bd^T @ wk  [rows b*8 + 2u+p, 512e] ============
                with tc.tile_pool(name="psQK", bufs=1, space="PSUM") as psQK:
                    pkr = psQK.tile([32, E], F32, tag="pkr")
                    pki = psQK.tile([32, E], F32, tag="pki")
                    for j, (bdt, w) in enumerate(
                            [(bd_r[u], wkr[u]) for u in range(4)]
                            + [(bd_in[u], wki[u]) for u in range(4)]):
                        mm(pkr[:], bdt[:], w[:], start=(j == 0), stop=(j == 7))
                    for j, (bdt, w) in enumerate(
                            [(bd_r[u], wki[u]) for u in range(4)]
                            + [(bd_i[u], wkr[u]) for u in range(4)]):
                        mm(pki[:], bdt[:], w[:], start=(j == 0), stop=(j == 7))
                    nc.vector.tensor_copy(qk_sb_r[:], pkr[:])
                    nc.scalar.copy(qk_sb_i[:], pki[:])

                # vC: token-256 v row for all batches (fills PE during qk copies)
                with tc.tile_pool(name="psVC", bufs=1, space="PSUM") as psVC:
                    p1 = psVC.tile([BPC, OUT], F32, tag="pc1")
                    pi = psVC.tile([BPC, OUT], F32, tag="pci")
                    for j, (x, w) in enumerate(
                            [(xbr[e][:, :, 256], wvr[e]) for e in range(4)]
                            + [(xbi[e][:, :, 256], wvin[e]) for e in range(4)]):
                        mm(p1[:], x, w[:], start=(j == 0), stop=(j == 7))
                    for j, (x, w) in enumerate(
                            [(xbr[e][:, :, 256], wvi[e]) for e in range(4)]
                            + [(xbi[e][:, :, 256], wvr[e]) for e in range(4)]):
                        mm(pi[:], x, w[:], start=(j == 0), stop=(j == 7))
                    nc.vector.tensor_copy(vCr_sb[:], p1[:])
                    nc.scalar.copy(vCi_sb[:], pi[:])

                # transpose qk -> qkT [128e, 4u, 8] (+ negated imag)
                with tc.tile_pool(name="psQT", bufs=2, space="PSUM") as psQT:
                    for e in range(4):
                        es = slice(e * 128, (e + 1) * 128)
                        ptr = psQT.tile([128, 32], F32, tag="qtr", name=f"qtr{e}")
                        pti = psQT.tile([128, 32], F32, tag="qti", name=f"qti{e}")
                        nc.tensor.transpose(ptr[:], qk_sb_r[:, es], ident[0:32, 0:32])
                        nc.tensor.transpose(pti[:], qk_sb_i[:, es], ident[0:32, 0:32])
                        nc.scalar.copy(qkT_r[e][:], ptr[:])
                        nc.scalar.copy(qkT_i[e][:], pti[:])
                        nc.vector.tensor_scalar_mul(qkT_in[e][:], pti[:], -1.0)

            # ============ logits [8, SP] per batch (row = 2u+p = h) and
            # softmax straight out of PSUM into per-batch w_b tiles ============
            # two passes (all-real then all-imag) so softmax overlaps PE
            with tc.tile_pool(name="miscB2", bufs=1) as mb:
                w_b = [mb.tile([8, 2, S], F32, name=f"w_b{b}") for b in range(BPC)]
                with tc.tile_pool(name="psB2", bufs=3, space="PSUM") as psB2:
                    def softmax(b, ri, psum):
                        # logits are O(+-8): exp safe in f32 without max-shift
                        sm = mb.tile([8, 1], F32, tag="ssm", name=f"sm{b}_{ri}")
                        rs = mb.tile([8, 1], F32, tag="srs", name=f"rs{b}_{ri}")
                        nc.scalar.activation(w_b[b][:, ri, :], psum[:, 0:S],
                                             ACTF.Exp, bias=0.0, scale=1.0,
                                             accum_out=sm[:])
                        nc.vector.reciprocal(rs[:], sm[:])
                        nc.vector.tensor_scalar_mul(w_b[b][:, ri, :],
                                                    w_b[b][:, ri, :], rs[:])

                    for b in range(BPC):
                        pr = psB2.tile([8, SP], F32, tag="plgr", name=f"plgr{b}")
                        for j, (q, x) in enumerate(
                                [(qkT_r[e][:, b * 8:b * 8 + 8], xbr[e][:, b, :])
                                 for e in range(4)]
                                + [(qkT_in[e][:, b * 8:b * 8 + 8], xbi[e][:, b, :])
                                   for e in range(4)]):
                            mm(pr[:], q, x, start=(j == 0), stop=(j == 7))
                        softmax(b, 0, pr)
                    for b in range(BPC):
                        pq = psB2.tile([8, SP], F32, tag="plgi", name=f"plgi{b}")
                        for j, (q, x) in enumerate(
                                [(qkT_r[e][:, b * 8:b * 8 + 8], xbi[e][:, b, :])
                                 for e in range(4)]
                                + [(qkT_i[e][:, b * 8:b * 8 + 8], xbr[e][:, b, :])
                                   for e in range(4)]):
                            mm(pq[:], q, x, start=(j == 0), stop=(j == 7))
                        softmax(b, 1, pq)

                # vC2[p, b, :]: rows (re, im); vC2s rows (im, re)
                vC2 = mb.tile([2, BPC, OUT], BF16)
                vC2s = mb.tile([2, BPC, OUT], BF16)
                nc.sync.dma_start(out=vC2[0:1, :, :], in_=vCr_sb[:])
                nc.sync.dma_start(out=vC2[1:2, :, :], in_=vCi_sb[:])
                nc.sync.dma_start(out=vC2s[0:1, :, :], in_=vCi_sb[:])
                nc.sync.dma_start(out=vC2s[1:2, :, :], in_=vCr_sb[:])

                # ---- transpose w -> wT [S-part, 32] (bf16); col = b*8 + h ----
                wTr = [mb.tile([128, 32], BF16, name=f"wTr{a}") for a in range(2)]
                wTi = [mb.tile([128, 32], BF16, name=f"wTi{a}") for a in range(2)]
                wTin = [mb.tile([128, 32], BF16, name=f"wTin{a}") for a in range(2)]
                wtc_a = mb.tile([2, 32], BF16)   # rows: wTr_c, -wTi_c
                wtc_b = mb.tile([2, 32], BF16)   # rows: wTr_c, wTi_c
                with tc.tile_pool(name="psB3", bufs=1, space="PSUM") as psB3:
                    pw = [[psB3.tile([128, 32], F32, tag=f"pw{a}{ri}",
                                     name=f"pw{a}{ri}")
                           for ri in range(2)] for a in range(2)]
                    ptc = psB3.tile([2, 32], F32, tag="ptc")
                    for b in range(BPC):
                        ocols = slice(b * 8, b * 8 + 8)
                        for a in range(2):
                            cs = slice(a * 128, (a + 1) * 128)
                            for ri in range(2):
                                nc.tensor.matmul(pw[a][ri][:, ocols],
                                                 w_b[b][:, ri, cs],
                                                 ident[0:8, 0:8],
                                                 is_transpose=True,
                                                 skip_group_check=True)
                        nc.tensor.matmul(ptc[:, ocols], w_b[b][:, :, 256],
                                         ident[0:8, 0:8], is_transpose=True,
                                         skip_group_check=True)
                    for a in range(2):
                        nc.scalar.copy(wTr[a][:], pw[a][0][:])
                        nc.scalar.copy(wTi[a][:], pw[a][1][:])
                        nc.scalar.activation(wTin[a][:], pw[a][1][:], ACTF.Copy,
                                             bias=0.0, scale=-1.0)
                    wtc_neg = mb.tile([2, 32], BF16)
                    nc.scalar.copy(wtc_b[:], ptc[:])
                    nc.scalar.activation(wtc_neg[:], ptc[:], ACTF.Copy,
                                         bias=0.0, scale=-1.0)
                    nc.sync.dma_start(out=wtc_a[0:1, :], in_=wtc_b[0:1, :])
                    nc.sync.dma_start(out=wtc_a[1:2, :], in_=wtc_neg[1:2, :])

                # ---- hv: per batch [8, 512] -> mask -> DMA into hvm_all ----
                hvm_r = [mb.tile([NH, OUT], BF16, name=f"hvm_r{b}") for b in range(BPC)]
                hvm_i = [mb.tile([NH, OUT], BF16, name=f"hvm_i{b}") for b in range(BPC)]
                hvm_all_r = mb.tile([32, OUT], BF16)
                hvm_all_i = mb.tile([32, OUT], BF16)
                with tc.tile_pool(name="psB4", bufs=2, space="PSUM") as psB4:
                    for b in range(BPC):
                        cols = slice(b * 8, b * 8 + 8)
                        ph_r = psB4.tile([NH, OUT], F32, tag="phr", name=f"phr{b}")
                        ph_i = psB4.tile([NH, OUT], F32, tag="phi", name=f"phi{b}")
                        mm(ph_r[:], wTr[0][:, cols], vr[b][0][:], start=True, stop=False)
                        mm(ph_r[:], wTr[1][:, cols], vr[b][1][:], start=False, stop=False)
                        mm(ph_r[:], wTin[0][:, cols], vi[b][0][:], start=False, stop=False)
                        mm(ph_r[:], wTin[1][:, cols], vi[b][1][:], start=False, stop=False)
                        mm(ph_r[:], wtc_a[:, cols], vC2[:, b, :], start=False, stop=True)
                        mm(ph_i[:], wTi[0][:, cols], vr[b][0][:], start=True, stop=False)
                        mm(ph_i[:], wTi[1][:, cols], vr[b][1][:], start=False, stop=False)
                        mm(ph_i[:], wTr[0][:, cols], vi[b][0][:], start=False, stop=False)
                        mm(ph_i[:], wTr[1][:, cols], vi[b][1][:], start=False, stop=False)
                        mm(ph_i[:], wtc_b[:, cols], vC2s[:, b, :], start=False, stop=True)
                        nc.vector.tensor_mul(hvm_r[b][:], ph_r[:], mask8[:])
                        nc.vector.tensor_mul(hvm_i[b][:], ph_i[:], mask8[:])
                        nc.sync.dma_start(out=hvm_all_r[b * 8:b * 8 + 8, :],
                                          in_=hvm_r[b][:])
                        nc.sync.dma_start(out=hvm_all_i[b * 8:b * 8 + 8, :],
                                          in_=hvm_i[b][:])

                # ---- extract attn0^T [128, 4] per f-tile via selection matmul ----
                att_r = [mb.tile([128, 4], BF16, name=f"att_r{u}") for u in range(4)]
                att_i = [mb.tile([128, 4], BF16, name=f"att_i{u}") for u in range(4)]
                att_in = [mb.tile([128, 4], BF16, name=f"att_in{u}") for u in range(4)]
                with tc.tile_pool(name="psB5", bufs=2, space="PSUM") as psB5:
                    for u in range(4):
                        fs = slice(u * 128, (u + 1) * 128)
                        par = psB5.tile([128, 4], F32, tag="par", name=f"par{u}")
                        pai = psB5.tile([128, 4], F32, tag="pai", name=f"pai{u}")
                        mm(par[:], hvm_all_r[:, fs], sel32[:], start=True, stop=True)
                        mm(pai[:], hvm_all_i[:, fs], sel32[:], start=True, stop=True)
                        nc.scalar.copy(att_r[u][:], par[:])
                        nc.scalar.copy(att_i[u][:], pai[:])
                        nc.scalar.activation(att_in[u][:], pai[:], ACTF.Copy,
                                             bias=0.0, scale=-1.0)

                # ---- y = attn0 @ Wc^T + b_c ----
                yr_sb = mb.tile([BPC, OUT], F32)
                yi_sb = mb.tile([BPC, OUT], F32)
                with tc.tile_pool(name="psB6", bufs=1, space="PSUM") as psB6:
                    py_r = psB6.tile([BPC, OUT], F32, tag="pyr")
                    py_i = psB6.tile([BPC, OUT], F32, tag="pyi")
                    for j, u in enumerate(range(4)):
                        mm(py_r[:], att_r[u][:], wcr[u][:], start=(j == 0), stop=False)
                        mm(py_r[:], att_in[u][:], wci[u][:], start=False, stop=(j == 3))
                        mm(py_i[:], att_r[u][:], wci[u][:], start=(j == 0), stop=False)
                        mm(py_i[:], att_i[u][:], wcr[u][:], start=False, stop=(j == 3))
                    nc.vector.tensor_add(yr_sb[:], py_r[:], bcr[:])
                    nc.vector.tensor_add(yi_sb[:], py_i[:], bci[:])
                    nc.sync.dma_start(out=d_yr.ap(), in_=yr_sb[:])
                    nc.sync.dma_start(out=d_yi.ap(), in_=yi_sb[:])

    nc.compile()
    return nc


def _host_prep(inputs):
    """Build per-core in_maps from the full inputs."""
    import ml_dtypes
    f32 = np.float32
    bf16 = ml_dtypes.bfloat16
    xr = np.ascontiguousarray(inputs["x_real"], dtype=f32).reshape(B, E, HW)
    xi = np.ascontiguousarray(inputs["x_imag"], dtype=f32).reshape(B, E, HW)
    pos = np.asarray(inputs["pos_r"], dtype=f32) + 1j * np.asarray(inputs["pos_i"], dtype=f32)
    w_in_r = np.asarray(inputs["w_in_r"], dtype=f32)
    w_in_i = np.asarray(inputs["w_in_i"], dtype=f32)
    b_in_r = np.asarray(inputs["b_in_r"], dtype=f32)
    b_in_i = np.asarray(inputs["b_in_i"], dtype=f32)
    w_out = np.asarray(inputs["w_out_r"], dtype=f32) + 1j * np.asarray(inputs["w_out_i"], dtype=f32)
    b_out = np.asarray(inputs["b_out_r"], dtype=f32) + 1j * np.asarray(inputs["b_out_i"], dtype=f32)
    w_p = np.asarray(inputs["w_p_r"], dtype=f32) + 1j * np.asarray(inputs["w_p_i"], dtype=f32)
    b_p = np.asarray(inputs["b_p_r"], dtype=f32) + 1j * np.asarray(inputs["b_p_i"], dtype=f32)

    w_in = w_in_r + 1j * w_in_i
    wq, wk, wv = w_in[:E], w_in[E:2 * E], w_in[2 * E:]
    qs = f32(1.0 / np.sqrt(HD))

    posb = np.zeros((E, SP), np.complex64)
    posb[:, :S] = pos

    wc = w_p @ w_out                                        # [OUT, E] complex
    bq = qs * (b_in_r[:E] + 1j * b_in_i[:E])                # [E]

    b_v = b_in_r[2 * E:] + 1j * b_in_i[2 * E:]
    b_c = (1 + 1j) * (b_v @ wc.T) + b_out @ w_p.T + b_p     # [OUT] complex

    mask8 = np.zeros((NH, OUT), f32)
    for h in range(NH):
        mask8[h, h * HD:(h + 1) * HD] = 1.0
    sel32 = np.zeros((32, BPC), f32)
    for b in range(BPC):
        sel32[b * 8:(b + 1) * 8, b] = 1.0

    shared = dict(
        wqr=np.ascontiguousarray(wq.real.T * qs).astype(bf16),
        wqi=np.ascontiguousarray(wq.imag.T * qs).astype(bf16),
        wkr=np.ascontiguousarray(wk.real).astype(bf16),
        wki=np.ascontiguousarray(wk.imag).astype(bf16),
        wvr=np.ascontiguousarray(wv.real.T).astype(bf16),
        wvi=np.ascontiguousarray(wv.imag.T).astype(bf16),
        wvin=np.ascontiguousarray(-wv.imag.T).astype(bf16),
        wcr=np.ascontiguousarray(wc.real.T.astype(f32)).astype(bf16),
        wci=np.ascontiguousarray(wc.imag.T.astype(f32)).astype(bf16),
        bqr=bq.real.astype(f32).reshape(4, 128).T.copy(),
        bqi=bq.imag.astype(f32).reshape(4, 128).T.copy(),
        bcr=np.broadcast_to(b_c.real.astype(f32), (BPC, OUT)).copy(),
        bci=np.broadcast_to(b_c.imag.astype(f32), (BPC, OUT)).copy(),
        ident=np.eye(128, dtype=f32),
        mask8=mask8,
        sel32=sel32.astype(bf16),
        zbd=np.zeros((128, 32), bf16),
    )
    # x_cat fully prepped on host: col 0 = mean, then + pos; col 257 zero
    xrp = np.zeros((B, E, SP), f32)
    xip = np.zeros((B, E, SP), f32)
    xrp[:, :, 1:1 + HW] = xr
    xip[:, :, 1:1 + HW] = xi
    xrp[:, :, 0] = xr.mean(-1)
    xip[:, :, 0] = xi.mean(-1)
    xrp[:, :, :S] += posb.real[None, :, :S]
    xip[:, :, :S] += posb.imag[None, :, :S]
    in_maps = []
    for c in range(NCORES):
        m = dict(shared)
        m["xr"] = np.ascontiguousarray(
            xrp[c * BPC:(c + 1) * BPC].transpose(1, 0, 2)).astype(bf16)
        m["xi"] = np.ascontiguousarray(
            xip[c * BPC:(c + 1) * BPC].transpose(1, 0, 2)).astype(bf16)
        in_maps.append(m)
    return in_maps


def _run(inputs, trace=False, **kw):
    from concourse.bass_utils import run_bass_kernel_spmd
    if "nc" not in _cached:
        _cached["nc"] = _build()
    nc = _cached["nc"]
    in_maps = _host_prep(inputs)
    res = run_bass_kernel_spmd(nc, in_maps, core_ids=list(range(NCORES)),
                               trace=trace, **kw)
    out = np.empty((B, OUT), np.complex64)
    for c in range(NCORES):
        out[c * BPC:(c + 1) * BPC] = (res.results[c]["yr"]
                                      + 1j * res.results[c]["yi"])
    return out, res


def kernel(**inputs) -> np.ndarray:
    out, _ = _run(inputs)
    return out


# revision 22
# speedup vs baseline: 1.7315x; 1.0937x over previous
"""Complex AttentionPool2d on 8 trn2 NeuronCores, data-parallel over batch.

Contract: kernel(**inputs) takes the FULL inputs from setup_inputs() and
returns the FULL [32, 512] complex64 output.

V2: all matmuls bf16 (fp32 PSUM accum); k^T eliminated algebraically.
Math (per batch):
  x = bf16(complex(x_real, x_imag)).reshape(E, 256)
  x_cat = [mean(x), x] + pos                       # [E, 257]
  q0 = x_cat[:, 0] @ wq^T + bq                     # only query pos 0 matters
  qk[h, e] = sum_d q0[h*64+d] wk[h*64+d, e]        # fold q into k-proj
  logits[h, s] = sum_e qk[h, e] x_cat[e, s]        # == q0 . k[s]
  w = softmax(logits.re) + i*softmax(logits.im)
  v = x_cat^T @ wv^T                               # [257, 512]
  attn0 = (w @ v) per-head masked; y = attn0 @ (w_p @ w_out)^T + b_c

Sharding: batch 32 -> 4 per core.
"""
import numpy as np

B, E, HW, S = 32, 512, 256, 257
SP = 258            # S padded even
NH, HD = 8, 64
OUT = 512
NCORES = 8
BPC = B // NCORES   # batches per core

_cached = {}


def _build():
    import concourse.bacc as bacc
    import concourse.tile as tile
    import concourse.mybir as mybir

    F32 = mybir.dt.float32
    BF16 = mybir.dt.bfloat16
    AX = mybir.AxisListType
    ACTF = mybir.ActivationFunctionType

    nc = bacc.Bacc("TRN2", target_bir_lowering=False, debug=False)

    # ---- DRAM I/O ----
    # x layout: [E, BPC, SP] so one DMA per e-tile covers all 4 batches;
    # col 0 reserved for the mean token, col 257 zero pad
    d_xr = nc.dram_tensor("xr", [E, BPC, SP], BF16, kind="ExternalInput")
    d_xi = nc.dram_tensor("xi", [E, BPC, SP], BF16, kind="ExternalInput")
    d_wqr = nc.dram_tensor("wqr", [E, E], BF16, kind="ExternalInput")
    d_wqi = nc.dram_tensor("wqi", [E, E], BF16, kind="ExternalInput")
    d_wkr = nc.dram_tensor("wkr", [E, E], BF16, kind="ExternalInput")
    d_wki = nc.dram_tensor("wki", [E, E], BF16, kind="ExternalInput")
    d_wvr = nc.dram_tensor("wvr", [E, OUT], BF16, kind="ExternalInput")
    d_wvi = nc.dram_tensor("wvi", [E, OUT], BF16, kind="ExternalInput")
    d_wcr = nc.dram_tensor("wcr", [E, OUT], BF16, kind="ExternalInput")
    d_wci = nc.dram_tensor("wci", [E, OUT], BF16, kind="ExternalInput")
    d_bqr = nc.dram_tensor("bqr", [128, 4], F32, kind="ExternalInput")
    d_bqi = nc.dram_tensor("bqi", [128, 4], F32, kind="ExternalInput")
    d_bcr = nc.dram_tensor("bcr", [BPC, OUT], F32, kind="ExternalInput")
    d_bci = nc.dram_tensor("bci", [BPC, OUT], F32, kind="ExternalInput")
    d_id = nc.dram_tensor("ident", [128, 128], F32, kind="ExternalInput")
    d_mask = nc.dram_tensor("mask8", [NH, OUT], F32, kind="ExternalInput")
    d_sel = nc.dram_tensor("sel32", [32, BPC], BF16, kind="ExternalInput")
    d_zbd = nc.dram_tensor("zbd", [128, 32], BF16, kind="ExternalInput")
    d_yr = nc.dram_tensor("yr", [BPC, OUT], F32, kind="ExternalOutput")
    d_yi = nc.dram_tensor("yi", [BPC, OUT], F32, kind="ExternalOutput")

    with tile.TileContext(nc) as tc:
        with tc.tile_pool(name="consts", bufs=1) as consts, \
             tc.tile_pool(name="vpool", bufs=1) as vpool:
            # ---- persistent weights / constants (bf16) ----
            wvr = [consts.tile([128, OUT], BF16, name=f"wvr{e}") for e in range(4)]
            wvi = [consts.tile([128, OUT], BF16, name=f"wvi{e}") for e in range(4)]
            wvin = [consts.tile([128, OUT], BF16, name=f"wvin{e}") for e in range(4)]
            wqr = [consts.tile([128, E], BF16, name=f"wqr{e}") for e in range(4)]
            wqi = [consts.tile([128, E], BF16, name=f"wqi{e}") for e in range(4)]
            wkr = [consts.tile([128, E], BF16, name=f"wkr{e}") for e in range(4)]
            wki = [consts.tile([128, E], BF16, name=f"wki{e}") for e in range(4)]
            wcr = [consts.tile([128, OUT], BF16, name=f"wcr{e}") for e in range(4)]
            wci = [consts.tile([128, OUT], BF16, name=f"wci{e}") for e in range(4)]
            bqr = consts.tile([128, 4], F32)
            bqi = consts.tile([128, 4], F32)
            bqin = consts.tile([128, 4], F32)
            bcr = consts.tile([BPC, OUT], F32)
            bci = consts.tile([BPC, OUT], F32)
            ident = consts.tile([128, 128], F32)
            mask8 = consts.tile([NH, OUT], F32)
            sel32 = consts.tile([32, BPC], BF16)

            # x tiles: [128e, BPC, SP]
            xbr = [vpool.tile([128, BPC, SP], BF16, name=f"xbr{e}") for e in range(4)]
            xbi = [vpool.tile([128, BPC, SP], BF16, name=f"xbi{e}") for e in range(4)]
            x0in = [vpool.tile([128, BPC], BF16, name=f"x0in{e}") for e in range(4)]
            # v tiles live until hv
            vr = [[vpool.tile([128, OUT], BF16, name=f"vr{b}_{s}")
                   for s in range(2)] for b in range(BPC)]
            vi = [[vpool.tile([128, OUT], BF16, name=f"vi{b}_{s}")
                   for s in range(2)] for b in range(BPC)]
            vCr_sb = vpool.tile([BPC, OUT], BF16)
            vCi_sb = vpool.tile([BPC, OUT], BF16)
            # bd: per-u zero-padded block-diag q0 [128, 32] (cols b*8+2u+p)
            bd_r = [vpool.tile([128, 32], BF16, name=f"bd_r{u}") for u in range(4)]
            bd_i = [vpool.tile([128, 32], BF16, name=f"bd_i{u}") for u in range(4)]
            bd_in = [vpool.tile([128, 32], BF16, name=f"bd_in{u}") for u in range(4)]
            q0r_sb = vpool.tile([BPC, E], F32)
            q0i_sb = vpool.tile([BPC, E], F32)
            qk_sb_r = vpool.tile([32, E], F32)
            qk_sb_i = vpool.tile([32, E], F32)
            qkT_r = [vpool.tile([128, 32], BF16, name=f"qkTr{e}") for e in range(4)]
            qkT_i = [vpool.tile([128, 32], BF16, name=f"qkTi{e}") for e in range(4)]
            qkT_in = [vpool.tile([128, 32], BF16, name=f"qkTin{e}") for e in range(4)]

            # ---- DMA issue order matters per queue ----
            # sync queue: small consts then x real
            nc.sync.dma_start(out=ident, in_=d_id.ap())
            nc.sync.dma_start(out=sel32, in_=d_sel.ap())
            nc.sync.dma_start(out=mask8, in_=d_mask.ap())
            nc.sync.dma_start(out=bqr, in_=d_bqr.ap())
            nc.sync.dma_start(out=bqi, in_=d_bqi.ap())
            nc.sync.dma_start(out=bcr, in_=d_bcr.ap())
            nc.sync.dma_start(out=bci, in_=d_bci.ap())
            for u in range(4):
                nc.sync.dma_start(out=bd_r[u], in_=d_zbd.ap())
                nc.sync.dma_start(out=bd_i[u], in_=d_zbd.ap())
                nc.sync.dma_start(out=bd_in[u], in_=d_zbd.ap())
            # hw queues interleaved in first-need order: v batch 0 consumes
            # (xbr[e], wvr[e]) then (xbi[e], wvin[e]) then wvi
            for e in range(4):
                sl = slice(e * 128, (e + 1) * 128)
                nc.scalar.dma_start(out=xbr[e][:], in_=d_xr.ap()[sl, :, :])
                nc.scalar.dma_start(out=wvr[e], in_=d_wvr.ap()[sl, :])
                nc.gpsimd.dma_start(out=xbi[e][:], in_=d_xi.ap()[sl, :, :])
                nc.gpsimd.dma_start(out=wvi[e], in_=d_wvi.ap()[sl, :])
            for e in range(4):
                sl = slice(e * 128, (e + 1) * 128)
                nc.scalar.dma_start(out=wqr[e], in_=d_wqr.ap()[sl, :])
                nc.scalar.dma_start(out=wqi[e], in_=d_wqi.ap()[sl, :])
            for e in range(4):
                sl = slice(e * 128, (e + 1) * 128)
                nc.gpsimd.dma_start(out=wkr[e], in_=d_wkr.ap()[sl, :])
                nc.gpsimd.dma_start(out=wki[e], in_=d_wki.ap()[sl, :])
            for e in range(4):
                sl = slice(e * 128, (e + 1) * 128)
                nc.gpsimd.dma_start(out=wcr[e], in_=d_wcr.ap()[sl, :])
                nc.gpsimd.dma_start(out=wci[e], in_=d_wci.ap()[sl, :])

            nc.vector.tensor_scalar_mul(bqin, bqi, -1.0)
            # negate wv imag on device (keeps it off the DMA critical path)
            for e in range(4):
                nc.vector.tensor_scalar_mul(wvin[e][:], wvi[e][:], -1.0)
            # x arrives fully prepped from host (mean in col 0, pos added);
            # only the negated imag of token 0 is built on device
            for e in range(4):
                nc.scalar.activation(x0in[e][:], xbi[e][:, :, 0], ACTF.Copy,
                                     bias=0.0, scale=-1.0)

            mm = nc.tensor.matmul

            with tc.tile_pool(name="psA", bufs=2, space="PSUM") as psA:
                # v rows s in [sb*128, (sb+1)*128) = x cols (col 0 = mean tok)
                def emit_v(b):
                    for sb in range(2):
                        cs = slice(sb * 128, (sb + 1) * 128)
                        p1 = psA.tile([128, OUT], F32, tag="pv1", name=f"pv1_{b}_{sb}")
                        pi = psA.tile([128, OUT], F32, tag="pvi", name=f"pvi_{b}_{sb}")
                        for j, (x, w) in enumerate(
                                [(xbr[e][:, b, cs], wvr[e]) for e in range(4)]
                                + [(xbi[e][:, b, cs], wvin[e]) for e in range(4)]):
                            mm(p1[:], x, w[:], start=(j == 0), stop=(j == 7))
                        for j, (x, w) in enumerate(
                                [(xbr[e][:, b, cs], wvi[e]) for e in range(4)]
                                + [(xbi[e][:, b, cs], wvr[e]) for e in range(4)]):
                            mm(pi[:], x, w[:], start=(j == 0), stop=(j == 7))
                        nc.vector.tensor_copy(vr[b][sb][:], p1[:])
                        nc.scalar.copy(vi[b][sb][:], pi[:])

                for b in range(3):
                    emit_v(b)

                # ============ q0 -> bd ============
                with tc.tile_pool(name="psB1", bufs=1, space="PSUM") as psB1:
                    pqr = psB1.tile([BPC, E], F32, tag="pqr")
                    pqi = psB1.tile([BPC, E], F32, tag="pqi")
                    for j, (x, w) in enumerate(
                            [(xbr[e][:, :, 0], wqr[e][:]) for e in range(4)]
                            + [(x0in[e][:], wqi[e][:]) for e in range(4)]):
                        mm(pqr[:], x, w, start=(j == 0), stop=(j == 7))
                    for j, (x, w) in enumerate(
                            [(xbr[e][:, :, 0], wqi[e][:]) for e in range(4)]
                            + [(xbi[e][:, :, 0], wqr[e][:]) for e in range(4)]):
                        mm(pqi[:], x, w, start=(j == 0), stop=(j == 7))
                    nc.scalar.copy(q0r_sb[:], pqr[:])
                    nc.scalar.copy(q0i_sb[:], pqi[:])

                    # transpose q0 -> bd block-diag [128, 4u, 8]
                    # bd[p*64+d, u, 2b+p] = q0[b, u*128+p*64+d] + bq bias
                    for u in range(4):
                        fs = slice(u * 128, (u + 1) * 128)
                        ptr = psB1.tile([128, 4], F32, tag="ptq", bufs=1, name=f"ptq{u}")
                        pti = psB1.tile([128, 4], F32, tag="ptj", bufs=1, name=f"ptj{u}")
                        nc.tensor.transpose(ptr[:], q0r_sb[:, fs], ident[0:BPC, 0:BPC])
                        nc.tensor.transpose(pti[:], q0i_sb[:, fs], ident[0:BPC, 0:BPC])
                        for p in range(2):
                            rows = slice(p * 64, (p + 1) * 64)
                            cols = slice(2 * u + p, 32, 8)
                            nc.scalar.activation(bd_r[u][rows, cols], ptr[rows, :],
                                                 ACTF.Identity,
                                                 bias=bqr[rows, u:u + 1], scale=1.0)
                            nc.scalar.activation(bd_i[u][rows, cols], pti[rows, :],
                                                 ACTF.Identity,
                                                 bias=bqi[rows, u:u + 1], scale=1.0)
                            nc.scalar.activation(bd_in[u][rows, cols], pti[rows, :],
                                                 ACTF.Identity,
                                                 bias=bqin[rows, u:u + 1], scale=-1.0)

                # last v batch fills PE while bd copies run
                emit_v(3)

                # ============ qk = bd^T @ wk  [rows b*8 + 2u+p, 512e] ============
                with tc.tile_pool(name="psQK", bufs=1, space="PSUM") as psQK:
                    pkr = psQK.tile([32, E], F32, tag="pkr")
                    pki = psQK.tile([32, E], F32, tag="pki")
                    for j, (bdt, w) in enumerate(
                            [(bd_r[u], wkr[u]) for u in range(4)]
                            + [(bd_in[u], wki[u]) for u in range(4)]):
                        mm(pkr[:], bdt[:], w[:], start=(j == 0), stop=(j == 7))
                    for j, (bdt, w) in enumerate(
                            [(bd_r[u], wki[u]) for u in range(4)]
                            + [(bd_i[u], wkr[u]) for u in range(4)]):
                        mm(pki[:], bdt[:], w[:], start=(j == 0), stop=(j == 7))
                    nc.vector.tensor_copy(qk_sb_r[:], pkr[:])
                    nc.scalar.copy(qk_sb_i[:], pki[:])

                # vC: token-256 v row for all batches (fills PE during qk copies)
                with tc.tile_pool(name="psVC", bufs=1, space="PSUM") as psVC:
                    p1 = psVC.tile([BPC, OUT], F32, tag="pc1")
                    pi = psVC.tile([BPC, OUT], F32, tag="pci")
                    for j, (x, w) in enumerate(
                            [(xbr[e][:, :, 256], wvr[e]) for e in range(4)]
                            + [(xbi[e][:, :, 256], wvin[e]) for e in range(4)]):
                        mm(p1[:], x, w[:], start=(j == 0), stop=(j == 7))
                    for j, (x, w) in enumerate(
                            [(xbr[e][:, :, 256], wvi[e]) for e in range(4)]
                            + [(xbi[e][:, :, 256], wvr[e]) for e in range(4)]):
                        mm(pi[:], x, w[:], start=(j == 0), stop=(j == 7))
                    nc.vector.tensor_copy(vCr_sb[:], p1[:])
                    nc.scalar.copy(vCi_sb[:], pi[:])

                # transpose qk -> qkT [128e, 4u, 8] (+ negated imag)
                with tc.tile_pool(name="psQT", bufs=2, space="PSUM") as psQT:
                    for e in range(4):
                        es = slice(e * 128, (e + 1) * 128)
                        ptr = psQT.tile([128, 32], F32, tag="qtr", name=f"qtr{e}")
                        pti = psQT.tile([128, 32], F32, tag="qti", name=f"qti{e}")
                        nc.tensor.transpose(ptr[:], qk_sb_r[:, es], ident[0:32, 0:32])
                        nc.tensor.transpose(pti[:], qk_sb_i[:, es], ident[0:32, 0:32])
                        nc.scalar.copy(qkT_r[e][:], ptr[:])
                        nc.scalar.copy(qkT_i[e][:], pti[:])
                        nc.vector.tensor_scalar_mul(qkT_in[e][:], pti[:], -1.0)

            # ============ per-batch pipeline: logits -> softmax -> wT -> hv
            # (hv of batch b overlaps logits of batch b+1 on PE) ============
            with tc.tile_pool(name="miscB2", bufs=1) as mb:
                # vC2[p, b, :]: rows (re, im); vC2s rows (im, re)
                vC2 = mb.tile([2, BPC, OUT], BF16)
                vC2s = mb.tile([2, BPC, OUT], BF16)
                nc.sync.dma_start(out=vC2[0:1, :, :], in_=vCr_sb[:])
                nc.sync.dma_start(out=vC2[1:2, :, :], in_=vCi_sb[:])
                nc.sync.dma_start(out=vC2s[0:1, :, :], in_=vCi_sb[:])
                nc.sync.dma_start(out=vC2s[1:2, :, :], in_=vCr_sb[:])

                w_b = [mb.tile([8, 2, S], F32, name=f"w_b{b}") for b in range(BPC)]
                wTr = [mb.tile([128, 32], BF16, name=f"wTr{a}") for a in range(2)]
                wTi = [mb.tile([128, 32], BF16, name=f"wTi{a}") for a in range(2)]
                wTin = [mb.tile([128, 32], BF16, name=f"wTin{a}") for a in range(2)]
                wtc_a = mb.tile([2, 32], BF16)   # rows: wTr_c, -wTi_c
                wtc_b = mb.tile([2, 32], BF16)   # rows: wTr_c, wTi_c
                wtc_neg = mb.tile([2, 32], BF16)
                hvm_r = [mb.tile([NH, OUT], BF16, name=f"hvm_r{b}") for b in range(BPC)]
                hvm_i = [mb.tile([NH, OUT], BF16, name=f"hvm_i{b}") for b in range(BPC)]
                hvm_all_r = mb.tile([32, OUT], BF16)
                hvm_all_i = mb.tile([32, OUT], BF16)

                with tc.tile_pool(name="psB2", bufs=2, space="PSUM") as psB2, \
                     tc.tile_pool(name="psB3", bufs=1, space="PSUM") as psB3, \
                     tc.tile_pool(name="psB4", bufs=2, space="PSUM") as psB4:
                    # pw[:, 0:2, :] = wT re s-halves; [:, 2:4, :] = im; ptc sep
                    pw = psB3.tile([128, 4, 32], F32, tag="pw")
                    ptc = psB3.tile([2, 32], F32, tag="ptc")

                    def softmax(b, ri, psum):
                        # logits are O(+-8): exp safe in f32 without max-shift
                        sm = mb.tile([8, 1], F32, tag="ssm", name=f"sm{b}_{ri}")
                        rs = mb.tile([8, 1], F32, tag="srs", name=f"rs{b}_{ri}")
                        nc.scalar.activation(w_b[b][:, ri, :], psum[:, 0:S],
                                             ACTF.Exp, bias=0.0, scale=1.0,
                                             accum_out=sm[:])
                        nc.vector.reciprocal(rs[:], sm[:])
                        nc.vector.tensor_scalar_mul(w_b[b][:, ri, :],
                                                    w_b[b][:, ri, :], rs[:])

                    for b in range(BPC):
                        bcols = slice(b * 8, b * 8 + 8)
                        # -- logits --
                        pr = psB2.tile([8, SP], F32, tag="plg", name=f"plgr{b}")
                        for j, (q, x) in enumerate(
                                [(qkT_r[e][:, bcols], xbr[e][:, b, :])
                                 for e in range(4)]
                                + [(qkT_in[e][:, bcols], xbi[e][:, b, :])
                                   for e in range(4)]):
                            mm(pr[:], q, x, start=(j == 0), stop=(j == 7))
                        softmax(b, 0, pr)
                        pq = psB2.tile([8, SP], F32, tag="plg", name=f"plgi{b}")
                        for j, (q, x) in enumerate(
                                [(qkT_r[e][:, bcols], xbi[e][:, b, :])
                                 for e in range(4)]
                                + [(qkT_i[e][:, bcols], xbr[e][:, b, :])
                                   for e in range(4)]):
                            mm(pq[:], q, x, start=(j == 0), stop=(j == 7))
                        softmax(b, 1, pq)
                        # -- transpose w -> wT columns for this batch --
                        for a in range(2):
                            cs = slice(a * 128, (a + 1) * 128)
                            for ri in range(2):
                                nc.tensor.matmul(pw[:, 2 * ri + a, bcols],
                                                 w_b[b][:, ri, cs],
                                                 ident[0:8, 0:8],
                                                 is_transpose=True,
                                                 skip_group_check=True)
                        nc.tensor.matmul(ptc[:, bcols], w_b[b][:, :, 256],
                                         ident[0:8, 0:8], is_transpose=True,
                                         skip_group_check=True)
                        for a in range(2):
                            nc.scalar.copy(wTr[a][:, bcols], pw[:, a, bcols])
                            nc.scalar.copy(wTi[a][:, bcols], pw[:, 2 + a, bcols])
                            nc.scalar.activation(wTin[a][:, bcols],
                                                 pw[:, 2 + a, bcols],
                                                 ACTF.Copy, bias=0.0, scale=-1.0)
                        nc.scalar.copy(wtc_b[:, bcols], ptc[:, bcols])
                        nc.scalar.activation(wtc_neg[:, bcols], ptc[:, bcols],
                                             ACTF.Copy, bias=0.0, scale=-1.0)
                        nc.sync.dma_start(out=wtc_a[0:1, bcols],
                                          in_=wtc_b[0:1, bcols])
                        nc.sync.dma_start(out=wtc_a[1:2, bcols],
                                          in_=wtc_neg[1:2, bcols])
                        # -- hv --
                        ph_r = psB4.tile([NH, OUT], F32, tag="phr", name=f"phr{b}")
                        ph_i = psB4.tile([NH, OUT], F32, tag="phi", name=f"phi{b}")
                        mm(ph_r[:], wTr[0][:, bcols], vr[b][0][:], start=True, stop=False)
                        mm(ph_r[:], wTr[1][:, bcols], vr[b][1][:], start=False, stop=False)
                        mm(ph_r[:], wTin[0][:, bcols], vi[b][0][:], start=False, stop=False)
                        mm(ph_r[:], wTin[1][:, bcols], vi[b][1][:], start=False, stop=False)
                        mm(ph_r[:], wtc_a[:, bcols], vC2[:, b, :], start=False, stop=True)
                        mm(ph_i[:], wTi[0][:, bcols], vr[b][0][:], start=True, stop=False)
                        mm(ph_i[:], wTi[1][:, bcols], vr[b][1][:], start=False, stop=False)
                        mm(ph_i[:], wTr[0][:, bcols], vi[b][0][:], start=False, stop=False)
                        mm(ph_i[:], wTr[1][:, bcols], vi[b][1][:], start=False, stop=False)
                        mm(ph_i[:], wtc_b[:, bcols], vC2s[:, b, :], start=False, stop=True)
                        nc.vector.tensor_mul(hvm_r[b][:], ph_r[:], mask8[:])
                        nc.vector.tensor_mul(hvm_i[b][:], ph_i[:], mask8[:])
                        nc.sync.dma_start(out=hvm_all_r[b * 8:b * 8 + 8, :],
                                          in_=hvm_r[b][:])
                        nc.sync.dma_start(out=hvm_all_i[b * 8:b * 8 + 8, :],
                                          in_=hvm_i[b][:])

                # ---- extract attn0^T [128, 4] per f-tile via selection matmul ----
                att_r = [mb.tile([128, 4], BF16, name=f"att_r{u}") for u in range(4)]
                att_i = [mb.tile([128, 4], BF16, name=f"att_i{u}") for u in range(4)]
                att_in = [mb.tile([128, 4], BF16, name=f"att_in{u}") for u in range(4)]
                with tc.tile_pool(name="psB5", bufs=2, space="PSUM") as psB5:
                    for u in range(4):
                        fs = slice(u * 128, (u + 1) * 128)
                        par = psB5.tile([128, 4], F32, tag="par", name=f"par{u}")
                        pai = psB5.tile([128, 4], F32, tag="pai", name=f"pai{u}")
                        mm(par[:], hvm_all_r[:, fs], sel32[:], start=True, stop=True)
                        mm(pai[:], hvm_all_i[:, fs], sel32[:], start=True, stop=True)
                        nc.scalar.copy(att_r[u][:], par[:])
                        nc.scalar.copy(att_i[u][:], pai[:])
                        nc.scalar.activation(att_in[u][:], pai[:], ACTF.Copy,
                                             bias=0.0, scale=-1.0)

                # ---- y = attn0 @ Wc^T + b_c ----
                yr_sb = mb.tile([BPC, OUT], F32)
                yi_sb = mb.tile([BPC, OUT], F32)
                with tc.tile_pool(name="psB6", bufs=1, space="PSUM") as psB6:
                    py_r = psB6.tile([BPC, OUT], F32, tag="pyr")
                    py_i = psB6.tile([BPC, OUT], F32, tag="pyi")
                    for j, u in enumerate(range(4)):
                        mm(py_r[:], att_r[u][:], wcr[u][:], start=(j == 0), stop=False)
                        mm(py_r[:], att_in[u][:], wci[u][:], start=False, stop=(j == 3))
                        mm(py_i[:], att_r[u][:], wci[u][:], start=(j == 0), stop=False)
                        mm(py_i[:], att_i[u][:], wcr[u][:], start=False, stop=(j == 3))
                    nc.vector.tensor_add(yr_sb[:], py_r[:], bcr[:])
                    nc.vector.tensor_add(yi_sb[:], py_i[:], bci[:])
                    nc.sync.dma_start(out=d_yr.ap(), in_=yr_sb[:])
                    nc.sync.dma_start(out=d_yi.ap(), in_=yi_sb[:])

    nc.compile()
    return nc


def _host_prep(inputs):
    """Build per-core in_maps from the full inputs."""
    import ml_dtypes
    f32 = np.float32
    bf16 = ml_dtypes.bfloat16
    xr = np.ascontiguousarray(inputs["x_real"], dtype=f32).reshape(B, E, HW)
    xi = np.ascontiguousarray(inputs["x_imag"], dtype=f32).reshape(B, E, HW)
    pos = np.asarray(inputs["pos_r"], dtype=f32) + 1j * np.asarray(inputs["pos_i"], dtype=f32)
    w_in_r = np.asarray(inputs["w_in_r"], dtype=f32)
    w_in_i = np.asarray(inputs["w_in_i"], dtype=f32)
    b_in_r = np.asarray(inputs["b_in_r"], dtype=f32)
    b_in_i = np.asarray(inputs["b_in_i"], dtype=f32)
    w_out = np.asarray(inputs["w_out_r"], dtype=f32) + 1j * np.asarray(inputs["w_out_i"], dtype=f32)
    b_out = np.asarray(inputs["b_out_r"], dtype=f32) + 1j * np.asarray(inputs["b_out_i"], dtype=f32)
    w_p = np.asarray(inputs["w_p_r"], dtype=f32) + 1j * np.asarray(inputs["w_p_i"], dtype=f32)
    b_p = np.asarray(inputs["b_p_r"], dtype=f32) + 1j * np.asarray(inputs["b_p_i"], dtype=f32)

    w_in = w_in_r + 1j * w_in_i
    wq, wk, wv = w_in[:E], w_in[E:2 * E], w_in[2 * E:]
    qs = f32(1.0 / np.sqrt(HD))

    posb = np.zeros((E, SP), np.complex64)
    posb[:, :S] = pos

    wc = w_p @ w_out                                        # [OUT, E] complex
    bq = qs * (b_in_r[:E] + 1j * b_in_i[:E])                # [E]

    b_v = b_in_r[2 * E:] + 1j * b_in_i[2 * E:]
    b_c = (1 + 1j) * (b_v @ wc.T) + b_out @ w_p.T + b_p     # [OUT] complex

    mask8 = np.zeros((NH, OUT), f32)
    for h in range(NH):
        mask8[h, h * HD:(h + 1) * HD] = 1.0
    sel32 = np.zeros((32, BPC), f32)
    for b in range(BPC):
        sel32[b * 8:(b + 1) * 8, b] = 1.0

    shared = dict(
        wqr=np.ascontiguousarray(wq.real.T * qs).astype(bf16),
        wqi=np.ascontiguousarray(wq.imag.T * qs).astype(bf16),
        wkr=np.ascontiguousarray(wk.real).astype(bf16),
        wki=np.ascontiguousarray(wk.imag).astype(bf16),
        wvr=np.ascontiguousarray(wv.real.T).astype(bf16),
        wvi=np.ascontiguousarray(wv.imag.T).astype(bf16),
        wcr=np.ascontiguousarray(wc.real.T.astype(f32)).astype(bf16),
        wci=np.ascontiguousarray(wc.imag.T.astype(f32)).astype(bf16),
        bqr=bq.real.astype(f32).reshape(4, 128).T.copy(),
        bqi=bq.imag.astype(f32).reshape(4, 128).T.copy(),
        bcr=np.broadcast_to(b_c.real.astype(f32), (BPC, OUT)).copy(),
        bci=np.broadcast_to(b_c.imag.astype(f32), (BPC, OUT)).copy(),
        ident=np.eye(128, dtype=f32),
        mask8=mask8,
        sel32=sel32.astype(bf16),
        zbd=np.zeros((128, 32), bf16),
    )
    # x_cat fully prepped on host: col 0 = mean, then + pos; col 257 zero
    xrp = np.zeros((B, E, SP), f32)
    xip = np.zeros((B, E, SP), f32)
    xrp[:, :, 1:1 + HW] = xr
    xip[:, :, 1:1 + HW] = xi
    xrp[:, :, 0] = xr.mean(-1)
    xip[:, :, 0] = xi.mean(-1)
    xrp[:, :, :S] += posb.real[None, :, :S]
    xip[:, :, :S] += posb.imag[None, :, :S]
    in_maps = []
    for c in range(NCORES):
        m = dict(shared)
        m["xr"] = np.ascontiguousarray(
            xrp[c * BPC:(c + 1) * BPC].transpose(1, 0, 2)).astype(bf16)
        m["xi"] = np.ascontiguousarray(
            xip[c * BPC:(c + 1) * BPC].transpose(1, 0, 2)).astype(bf16)
        in_maps.append(m)
    return in_maps


def _run(inputs, trace=False, **kw):
    from concourse.bass_utils import run_bass_kernel_spmd
    if "nc" not in _cached:
        _cached["nc"] = _build()
    nc = _cached["nc"]
    in_maps = _host_prep(inputs)
    res = run_bass_kernel_spmd(nc, in_maps, core_ids=list(range(NCORES)),
                               trace=trace, **kw)
    out = np.empty((B, OUT), np.complex64)
    for c in range(NCORES):
        out[c * BPC:(c + 1) * BPC] = (res.results[c]["yr"]
                                      + 1j * res.results[c]["yi"])
    return out, res


def kernel(**inputs) -> np.ndarray:
    out, _ = _run(inputs)
    return out


# revision 25
# speedup vs baseline: 1.7544x; 1.0132x over previous
"""Complex AttentionPool2d on 8 trn2 NeuronCores, data-parallel over batch.

Contract: kernel(**inputs) takes the FULL inputs from setup_inputs() and
returns the FULL [32, 512] complex64 output.

V2: all matmuls bf16 (fp32 PSUM accum); k^T eliminated algebraically.
Math (per batch):
  x = bf16(complex(x_real, x_imag)).reshape(E, 256)
  x_cat = [mean(x), x] + pos                       # [E, 257]
  q0 = x_cat[:, 0] @ wq^T + bq                     # only query pos 0 matters
  qk[h, e] = sum_d q0[h*64+d] wk[h*64+d, e]        # fold q into k-proj
  logits[h, s] = sum_e qk[h, e] x_cat[e, s]        # == q0 . k[s]
  w = softmax(logits.re) + i*softmax(logits.im)
  v = x_cat^T @ wv^T                               # [257, 512]
  attn0 = (w @ v) per-head masked; y = attn0 @ (w_p @ w_out)^T + b_c

Sharding: batch 32 -> 4 per core.
"""
import numpy as np

B, E, HW, S = 32, 512, 256, 257
SP = 258            # S padded even
NH, HD = 8, 64
OUT = 512
NCORES = 8
BPC = B // NCORES   # batches per core

_cached = {}


def _build():
    import concourse.bacc as bacc
    import concourse.tile as tile
    import concourse.mybir as mybir

    F32 = mybir.dt.float32
    BF16 = mybir.dt.bfloat16
    AX = mybir.AxisListType
    ACTF = mybir.ActivationFunctionType

    nc = bacc.Bacc("TRN2", target_bir_lowering=False, debug=False)

    # ---- DRAM I/O ----
    # x layout: [E, BPC, SP] so one DMA per e-tile covers all 4 batches;
    # col 0 reserved for the mean token, col 257 zero pad
    d_xr = nc.dram_tensor("xr", [E, BPC, SP], BF16, kind="ExternalInput")
    d_xi = nc.dram_tensor("xi", [E, BPC, SP], BF16, kind="ExternalInput")
    d_wqr = nc.dram_tensor("wqr", [E, E], BF16, kind="ExternalInput")
    d_wqi = nc.dram_tensor("wqi", [E, E], BF16, kind="ExternalInput")
    d_wkr = nc.dram_tensor("wkr", [E, E], BF16, kind="ExternalInput")
    d_wki = nc.dram_tensor("wki", [E, E], BF16, kind="ExternalInput")
    d_wvr = nc.dram_tensor("wvr", [E, OUT], BF16, kind="ExternalInput")
    d_wvi = nc.dram_tensor("wvi", [E, OUT], BF16, kind="ExternalInput")
    d_wcr = nc.dram_tensor("wcr", [E, OUT], BF16, kind="ExternalInput")
    d_wci = nc.dram_tensor("wci", [E, OUT], BF16, kind="ExternalInput")
    d_bqr = nc.dram_tensor("bqr", [128, 4], F32, kind="ExternalInput")
    d_bqi = nc.dram_tensor("bqi", [128, 4], F32, kind="ExternalInput")
    d_bcr = nc.dram_tensor("bcr", [BPC, OUT], F32, kind="ExternalInput")
    d_bci = nc.dram_tensor("bci", [BPC, OUT], F32, kind="ExternalInput")
    d_id = nc.dram_tensor("ident", [128, 128], F32, kind="ExternalInput")
    d_mask = nc.dram_tensor("mask8", [NH, OUT], F32, kind="ExternalInput")
    d_sel = nc.dram_tensor("sel32", [32, BPC], BF16, kind="ExternalInput")
    d_zbd = nc.dram_tensor("zbd", [128, 32], BF16, kind="ExternalInput")
    d_yr = nc.dram_tensor("yr", [BPC, OUT], F32, kind="ExternalOutput")
    d_yi = nc.dram_tensor("yi", [BPC, OUT], F32, kind="ExternalOutput")

    with tile.TileContext(nc) as tc:
        with tc.tile_pool(name="consts", bufs=1) as consts, \
             tc.tile_pool(name="vpool", bufs=1) as vpool:
            # ---- persistent weights / constants (bf16) ----
            wvr = [consts.tile([128, OUT], BF16, name=f"wvr{e}") for e in range(4)]
            wvi = [consts.tile([128, OUT], BF16, name=f"wvi{e}") for e in range(4)]
            wvin = [consts.tile([128, OUT], BF16, name=f"wvin{e}") for e in range(4)]
            wqr = [consts.tile([128, E], BF16, name=f"wqr{e}") for e in range(4)]
            wqi = [consts.tile([128, E], BF16, name=f"wqi{e}") for e in range(4)]
            wkr = [consts.tile([128, E], BF16, name=f"wkr{e}") for e in range(4)]
            wki = [consts.tile([128, E], BF16, name=f"wki{e}") for e in range(4)]
            wcr = [consts.tile([128, OUT], BF16, name=f"wcr{e}") for e in range(4)]
            wci = [consts.tile([128, OUT], BF16, name=f"wci{e}") for e in range(4)]
            bqr = consts.tile([128, 4], F32)
            bqi = consts.tile([128, 4], F32)
            bqin = consts.tile([128, 4], F32)
            bcr = consts.tile([BPC, OUT], F32)
            bci = consts.tile([BPC, OUT], F32)
            ident = consts.tile([128, 128], F32)
            mask8 = consts.tile([NH, OUT], F32)
            sel32 = consts.tile([32, BPC], BF16)

            # x tiles: [128e, BPC, SP]
            xbr = [vpool.tile([128, BPC, SP], BF16, name=f"xbr{e}") for e in range(4)]
            xbi = [vpool.tile([128, BPC, SP], BF16, name=f"xbi{e}") for e in range(4)]
            x0in = [vpool.tile([128, BPC], BF16, name=f"x0in{e}") for e in range(4)]
            # v tiles live until hv
            vr = [[vpool.tile([128, OUT], BF16, name=f"vr{b}_{s}")
                   for s in range(2)] for b in range(BPC)]
            vi = [[vpool.tile([128, OUT], BF16, name=f"vi{b}_{s}")
                   for s in range(2)] for b in range(BPC)]
            vCr_sb = vpool.tile([BPC, OUT], BF16)
            vCi_sb = vpool.tile([BPC, OUT], BF16)
            # bd: per-u zero-padded block-diag q0 [128, 32] (cols b*8+2u+p)
            bd_r = [vpool.tile([128, 32], BF16, name=f"bd_r{u}") for u in range(4)]
            bd_i = [vpool.tile([128, 32], BF16, name=f"bd_i{u}") for u in range(4)]
            bd_in = [vpool.tile([128, 32], BF16, name=f"bd_in{u}") for u in range(4)]
            q0r_sb = vpool.tile([BPC, E], F32)
            q0i_sb = vpool.tile([BPC, E], F32)
            qk_sb_r = vpool.tile([32, E], F32)
            qk_sb_i = vpool.tile([32, E], F32)
            qkT_r = [vpool.tile([128, 32], BF16, name=f"qkTr{e}") for e in range(4)]
            qkT_i = [vpool.tile([128, 32], BF16, name=f"qkTi{e}") for e in range(4)]
            qkT_in = [vpool.tile([128, 32], BF16, name=f"qkTin{e}") for e in range(4)]

            # ---- DMA issue order matters per queue ----
            # sync queue: small consts then x real
            nc.sync.dma_start(out=ident, in_=d_id.ap())
            nc.sync.dma_start(out=sel32, in_=d_sel.ap())
            nc.sync.dma_start(out=mask8, in_=d_mask.ap())
            nc.sync.dma_start(out=bqr, in_=d_bqr.ap())
            nc.sync.dma_start(out=bqi, in_=d_bqi.ap())
            nc.sync.dma_start(out=bcr, in_=d_bcr.ap())
            nc.sync.dma_start(out=bci, in_=d_bci.ap())
            for u in range(4):
                nc.sync.dma_start(out=bd_r[u], in_=d_zbd.ap())
                nc.sync.dma_start(out=bd_i[u], in_=d_zbd.ap())
                nc.sync.dma_start(out=bd_in[u], in_=d_zbd.ap())
            # hw queues interleaved in first-need order.
            # A (scalar): xr+wvr (v b0 real terms) with wq woven in, wc last.
            # B (gpsimd): xi+wvi (v imag terms), then wk.
            sl4 = [slice(e * 128, (e + 1) * 128) for e in range(4)]
            for e in range(4):
                nc.scalar.dma_start(out=xbr[e][:], in_=d_xr.ap()[sl4[e], :, :])
                nc.scalar.dma_start(out=wvr[e], in_=d_wvr.ap()[sl4[e], :])
                if e == 0:
                    nc.scalar.dma_start(out=wqr[0], in_=d_wqr.ap()[sl4[0], :])
                nc.gpsimd.dma_start(out=xbi[e][:], in_=d_xi.ap()[sl4[e], :, :])
                nc.gpsimd.dma_start(out=wvi[e], in_=d_wvi.ap()[sl4[e], :])
            for e in range(4):
                if e > 0:
                    nc.scalar.dma_start(out=wqr[e], in_=d_wqr.ap()[sl4[e], :])
                nc.scalar.dma_start(out=wqi[e], in_=d_wqi.ap()[sl4[e], :])
            for e in range(4):
                nc.gpsimd.dma_start(out=wkr[e], in_=d_wkr.ap()[sl4[e], :])
                nc.gpsimd.dma_start(out=wki[e], in_=d_wki.ap()[sl4[e], :])
            for e in range(4):
                nc.scalar.dma_start(out=wcr[e], in_=d_wcr.ap()[sl4[e], :])
                nc.gpsimd.dma_start(out=wci[e], in_=d_wci.ap()[sl4[e], :])

            nc.vector.tensor_scalar_mul(bqin, bqi, -1.0)
            # negate wv imag on device (keeps it off the DMA critical path)
            for e in range(4):
                nc.vector.tensor_scalar_mul(wvin[e][:], wvi[e][:], -1.0)
            # x arrives fully prepped from host (mean in col 0, pos added);
            # only the negated imag of token 0 is built on device
            for e in range(4):
                nc.scalar.activation(x0in[e][:], xbi[e][:, :, 0], ACTF.Copy,
                                     bias=0.0, scale=-1.0)

            mm = nc.tensor.matmul

            with tc.tile_pool(name="psA", bufs=2, space="PSUM") as psA:
                # v rows s in [sb*128, (sb+1)*128) = x cols (col 0 = mean tok)
                def emit_v(b):
                    for sb in range(2):
                        cs = slice(sb * 128, (sb + 1) * 128)
                        p1 = psA.tile([128, OUT], F32, tag="pv1", name=f"pv1_{b}_{sb}")
                        pi = psA.tile([128, OUT], F32, tag="pvi", name=f"pvi_{b}_{sb}")
                        for j, (x, w) in enumerate(
                                [(xbr[e][:, b, cs], wvr[e]) for e in range(4)]
                                + [(xbi[e][:, b, cs], wvin[e]) for e in range(4)]):
                            mm(p1[:], x, w[:], start=(j == 0), stop=(j == 7))
                        for j, (x, w) in enumerate(
                                [(xbr[e][:, b, cs], wvi[e]) for e in range(4)]
                                + [(xbi[e][:, b, cs], wvr[e]) for e in range(4)]):
                            mm(pi[:], x, w[:], start=(j == 0), stop=(j == 7))
                        nc.vector.tensor_copy(vr[b][sb][:], p1[:])
                        nc.scalar.copy(vi[b][sb][:], pi[:])

                emit_v(0)

                # ============ q0 -> bd ============
                with tc.tile_pool(name="psB1", bufs=1, space="PSUM") as psB1:
                    pqr = psB1.tile([BPC, E], F32, tag="pqr")
                    pqi = psB1.tile([BPC, E], F32, tag="pqi")
                    for j, (x, w) in enumerate(
                            [(xbr[e][:, :, 0], wqr[e][:]) for e in range(4)]
                            + [(x0in[e][:], wqi[e][:]) for e in range(4)]):
                        mm(pqr[:], x, w, start=(j == 0), stop=(j == 7))
                    for j, (x, w) in enumerate(
                            [(xbr[e][:, :, 0], wqi[e][:]) for e in range(4)]
                            + [(xbi[e][:, :, 0], wqr[e][:]) for e in range(4)]):
                        mm(pqi[:], x, w, start=(j == 0), stop=(j == 7))
                    nc.scalar.copy(q0r_sb[:], pqr[:])
                    nc.scalar.copy(q0i_sb[:], pqi[:])

                    # transpose q0 -> bd block-diag [128, 4u, 8]
                    # bd[p*64+d, u, 2b+p] = q0[b, u*128+p*64+d] + bq bias
                    for u in range(4):
                        fs = slice(u * 128, (u + 1) * 128)
                        ptr = psB1.tile([128, 4], F32, tag="ptq", bufs=1, name=f"ptq{u}")
                        pti = psB1.tile([128, 4], F32, tag="ptj", bufs=1, name=f"ptj{u}")
                        nc.tensor.transpose(ptr[:], q0r_sb[:, fs], ident[0:BPC, 0:BPC])
                        nc.tensor.transpose(pti[:], q0i_sb[:, fs], ident[0:BPC, 0:BPC])
                        for p in range(2):
                            rows = slice(p * 64, (p + 1) * 64)
                            cols = slice(2 * u + p, 32, 8)
                            nc.scalar.activation(bd_r[u][rows, cols], ptr[rows, :],
                                                 ACTF.Identity,
                                                 bias=bqr[rows, u:u + 1], scale=1.0)
                            nc.scalar.activation(bd_i[u][rows, cols], pti[rows, :],
                                                 ACTF.Identity,
                                                 bias=bqi[rows, u:u + 1], scale=1.0)
                            nc.scalar.activation(bd_in[u][rows, cols], pti[rows, :],
                                                 ACTF.Identity,
                                                 bias=bqin[rows, u:u + 1], scale=-1.0)

                # next v batch fills PE while bd copies run
                emit_v(1)

                # ============ qk = bd^T @ wk  [rows b*8 + 2u+p, 512e] ============
                with tc.tile_pool(name="psQK", bufs=1, space="PSUM") as psQK:
                    pkr = psQK.tile([32, E], F32, tag="pkr")
                    pki = psQK.tile([32, E], F32, tag="pki")
                    for j, (bdt, w) in enumerate(
                            [(bd_r[u], wkr[u]) for u in range(4)]
                            + [(bd_in[u], wki[u]) for u in range(4)]):
                        mm(pkr[:], bdt[:], w[:], start=(j == 0), stop=(j == 7))
                    for j, (bdt, w) in enumerate(
                            [(bd_r[u], wki[u]) for u in range(4)]
                            + [(bd_i[u], wkr[u]) for u in range(4)]):
                        mm(pki[:], bdt[:], w[:], start=(j == 0), stop=(j == 7))
                    nc.vector.tensor_copy(qk_sb_r[:], pkr[:])
                    nc.scalar.copy(qk_sb_i[:], pki[:])

                # v b2 + vC fill PE during qk copies / qkT transposes
                emit_v(2)

                # vC: token-256 v row for all batches
                with tc.tile_pool(name="psVC", bufs=1, space="PSUM") as psVC:
                    p1 = psVC.tile([BPC, OUT], F32, tag="pc1")
                    pi = psVC.tile([BPC, OUT], F32, tag="pci")
                    for j, (x, w) in enumerate(
                            [(xbr[e][:, :, 256], wvr[e]) for e in range(4)]
                            + [(xbi[e][:, :, 256], wvin[e]) for e in range(4)]):
                        mm(p1[:], x, w[:], start=(j == 0), stop=(j == 7))
                    for j, (x, w) in enumerate(
                            [(xbr[e][:, :, 256], wvi[e]) for e in range(4)]
                            + [(xbi[e][:, :, 256], wvr[e]) for e in range(4)]):
                        mm(pi[:], x, w[:], start=(j == 0), stop=(j == 7))
                    nc.vector.tensor_copy(vCr_sb[:], p1[:])
                    nc.scalar.copy(vCi_sb[:], pi[:])

                # transpose qk -> qkT [128e, 4u, 8] (+ negated imag)
                with tc.tile_pool(name="psQT", bufs=2, space="PSUM") as psQT:
                    for e in range(4):
                        es = slice(e * 128, (e + 1) * 128)
                        ptr = psQT.tile([128, 32], F32, tag="qtr", name=f"qtr{e}")
                        pti = psQT.tile([128, 32], F32, tag="qti", name=f"qti{e}")
                        nc.tensor.transpose(ptr[:], qk_sb_r[:, es], ident[0:32, 0:32])
                        nc.tensor.transpose(pti[:], qk_sb_i[:, es], ident[0:32, 0:32])
                        nc.scalar.copy(qkT_r[e][:], ptr[:])
                        nc.scalar.copy(qkT_i[e][:], pti[:])
                        nc.vector.tensor_scalar_mul(qkT_in[e][:], pti[:], -1.0)

                # last v batch fills PE while qkT copies land
                emit_v(3)

            # ============ per-batch pipeline: logits -> softmax -> wT -> hv
            # (hv of batch b overlaps logits of batch b+1 on PE) ============
            with tc.tile_pool(name="miscB2", bufs=1) as mb:
                # vC2[p, b, :]: rows (re, im); vC2s rows (im, re)
                vC2 = mb.tile([2, BPC, OUT], BF16)
                vC2s = mb.tile([2, BPC, OUT], BF16)
                nc.sync.dma_start(out=vC2[0:1, :, :], in_=vCr_sb[:])
                nc.sync.dma_start(out=vC2[1:2, :, :], in_=vCi_sb[:])
                nc.sync.dma_start(out=vC2s[0:1, :, :], in_=vCi_sb[:])
                nc.sync.dma_start(out=vC2s[1:2, :, :], in_=vCr_sb[:])

                w_b = [mb.tile([8, 2, S], F32, name=f"w_b{b}") for b in range(BPC)]
                wTr = [mb.tile([128, 32], BF16, name=f"wTr{a}") for a in range(2)]
                wTi = [mb.tile([128, 32], BF16, name=f"wTi{a}") for a in range(2)]
                wTin = [mb.tile([128, 32], BF16, name=f"wTin{a}") for a in range(2)]
                wtc_a = mb.tile([2, 32], BF16)   # rows: wTr_c, -wTi_c
                wtc_b = mb.tile([2, 32], BF16)   # rows: wTr_c, wTi_c
                wtc_neg = mb.tile([2, 32], BF16)
                hvm_r = [mb.tile([NH, OUT], BF16, name=f"hvm_r{b}") for b in range(BPC)]
                hvm_i = [mb.tile([NH, OUT], BF16, name=f"hvm_i{b}") for b in range(BPC)]
                hvm_all_r = mb.tile([32, OUT], BF16)
                hvm_all_i = mb.tile([32, OUT], BF16)

                with tc.tile_pool(name="psB2", bufs=2, space="PSUM") as psB2, \
                     tc.tile_pool(name="psB3", bufs=1, space="PSUM") as psB3, \
                     tc.tile_pool(name="psB4", bufs=2, space="PSUM") as psB4:
                    # pw[:, 0:2, :] = wT re s-halves; [:, 2:4, :] = im; ptc sep
                    pw = psB3.tile([128, 4, 32], F32, tag="pw")
                    ptc = psB3.tile([2, 32], F32, tag="ptc")

                    def softmax(b, ri, psum):
                        # logits are O(+-8): exp safe in f32 without max-shift
                        sm = mb.tile([8, 1], F32, tag="ssm", name=f"sm{b}_{ri}")
                        rs = mb.tile([8, 1], F32, tag="srs", name=f"rs{b}_{ri}")
                        nc.scalar.activation(w_b[b][:, ri, :], psum[:, 0:S],
                                             ACTF.Exp, bias=0.0, scale=1.0,
                                             accum_out=sm[:])
                        nc.vector.reciprocal(rs[:], sm[:])
                        nc.vector.tensor_scalar_mul(w_b[b][:, ri, :],
                                                    w_b[b][:, ri, :], rs[:])

                    def emit_logits(b):
                        bcols = slice(b * 8, b * 8 + 8)
                        pr = psB2.tile([8, SP], F32, tag="plg", name=f"plgr{b}")
                        for j, (q, x) in enumerate(
                                [(qkT_r[e][:, bcols], xbr[e][:, b, :])
                                 for e in range(4)]
                                + [(qkT_in[e][:, bcols], xbi[e][:, b, :])
                                   for e in range(4)]):
                            mm(pr[:], q, x, start=(j == 0), stop=(j == 7))
                        softmax(b, 0, pr)
                        pq = psB2.tile([8, SP], F32, tag="plg", name=f"plgi{b}")
                        for j, (q, x) in enumerate(
                                [(qkT_r[e][:, bcols], xbi[e][:, b, :])
                                 for e in range(4)]
                                + [(qkT_i[e][:, bcols], xbr[e][:, b, :])
                                   for e in range(4)]):
                            mm(pq[:], q, x, start=(j == 0), stop=(j == 7))
                        softmax(b, 1, pq)

                    def emit_attn(b):
                        bcols = slice(b * 8, b * 8 + 8)
                        # -- transpose w -> wT columns for this batch --
                        for a in range(2):
                            cs = slice(a * 128, (a + 1) * 128)
                            for ri in range(2):
                                nc.tensor.matmul(pw[:, 2 * ri + a, bcols],
                                                 w_b[b][:, ri, cs],
                                                 ident[0:8, 0:8],
                                                 is_transpose=True,
                                                 skip_group_check=True)
                        nc.tensor.matmul(ptc[:, bcols], w_b[b][:, :, 256],
                                         ident[0:8, 0:8], is_transpose=True,
                                         skip_group_check=True)
                        for a in range(2):
                            nc.scalar.copy(wTr[a][:, bcols], pw[:, a, bcols])
                            nc.scalar.copy(wTi[a][:, bcols], pw[:, 2 + a, bcols])
                            nc.scalar.activation(wTin[a][:, bcols],
                                                 pw[:, 2 + a, bcols],
                                                 ACTF.Copy, bias=0.0, scale=-1.0)
                        nc.scalar.copy(wtc_b[:, bcols], ptc[:, bcols])
                        nc.scalar.activation(wtc_neg[:, bcols], ptc[:, bcols],
                                             ACTF.Copy, bias=0.0, scale=-1.0)
                        nc.sync.dma_start(out=wtc_a[0:1, bcols],
                                          in_=wtc_b[0:1, bcols])
                        nc.sync.dma_start(out=wtc_a[1:2, bcols],
                                          in_=wtc_neg[1:2, bcols])
                        # -- hv --
                        ph_r = psB4.tile([NH, OUT], F32, tag="phr", name=f"phr{b}")
                        ph_i = psB4.tile([NH, OUT], F32, tag="phi", name=f"phi{b}")
                        mm(ph_r[:], wTr[0][:, bcols], vr[b][0][:], start=True, stop=False)
                        mm(ph_r[:], wTr[1][:, bcols], vr[b][1][:], start=False, stop=False)
                        mm(ph_r[:], wTin[0][:, bcols], vi[b][0][:], start=False, stop=False)
                        mm(ph_r[:], wTin[1][:, bcols], vi[b][1][:], start=False, stop=False)
                        mm(ph_r[:], wtc_a[:, bcols], vC2[:, b, :], start=False, stop=True)
                        mm(ph_i[:], wTi[0][:, bcols], vr[b][0][:], start=True, stop=False)
                        mm(ph_i[:], wTi[1][:, bcols], vr[b][1][:], start=False, stop=False)
                        mm(ph_i[:], wTr[0][:, bcols], vi[b][0][:], start=False, stop=False)
                        mm(ph_i[:], wTr[1][:, bcols], vi[b][1][:], start=False, stop=False)
                        mm(ph_i[:], wtc_b[:, bcols], vC2s[:, b, :], start=False, stop=True)
                        nc.vector.tensor_mul(hvm_r[b][:], ph_r[:], mask8[:])
                        nc.vector.tensor_mul(hvm_i[b][:], ph_i[:], mask8[:])
                        nc.sync.dma_start(out=hvm_all_r[b * 8:b * 8 + 8, :],
                                          in_=hvm_r[b][:])
                        nc.sync.dma_start(out=hvm_all_i[b * 8:b * 8 + 8, :],
                                          in_=hvm_i[b][:])

                    # software-pipelined: logits of b+1 issue before attn of b
                    # so PE never waits on softmax
                    emit_logits(0)
                    emit_logits(1)
                    emit_attn(0)
                    emit_logits(2)
                    emit_attn(1)
                    emit_logits(3)
                    emit_attn(2)
                    emit_attn(3)

                # ---- extract attn0^T [128, 4] per f-tile via selection matmul ----
                att_r = [mb.tile([128, 4], BF16, name=f"att_r{u}") for u in range(4)]
                att_i = [mb.tile([128, 4], BF16, name=f"att_i{u}") for u in range(4)]
                att_in = [mb.tile([128, 4], BF16, name=f"att_in{u}") for u in range(4)]
                with tc.tile_pool(name="psB5", bufs=2, space="PSUM") as psB5:
                    for u in range(4):
                        fs = slice(u * 128, (u + 1) * 128)
                        par = psB5.tile([128, 4], F32, tag="par", name=f"par{u}")
                        pai = psB5.tile([128, 4], F32, tag="pai", name=f"pai{u}")
                        mm(par[:], hvm_all_r[:, fs], sel32[:], start=True, stop=True)
                        mm(pai[:], hvm_all_i[:, fs], sel32[:], start=True, stop=True)
                        nc.scalar.copy(att_r[u][:], par[:])
                        nc.scalar.copy(att_i[u][:], pai[:])
                        nc.scalar.activation(att_in[u][:], pai[:], ACTF.Copy,
                                             bias=0.0, scale=-1.0)

                # ---- y = attn0 @ Wc^T + b_c ----
                yr_sb = mb.tile([BPC, OUT], F32)
                yi_sb = mb.tile([BPC, OUT], F32)
                with tc.tile_pool(name="psB6", bufs=1, space="PSUM") as psB6:
                    py_r = psB6.tile([BPC, OUT], F32, tag="pyr")
                    py_i = psB6.tile([BPC, OUT], F32, tag="pyi")
                    for j, u in enumerate(range(4)):
                        mm(py_r[:], att_r[u][:], wcr[u][:], start=(j == 0), stop=False)
                        mm(py_r[:], att_in[u][:], wci[u][:], start=False, stop=(j == 3))
                        mm(py_i[:], att_r[u][:], wci[u][:], start=(j == 0), stop=False)
                        mm(py_i[:], att_i[u][:], wcr[u][:], start=False, stop=(j == 3))
                    nc.vector.tensor_add(yr_sb[:], py_r[:], bcr[:])
                    nc.vector.tensor_add(yi_sb[:], py_i[:], bci[:])
                    nc.sync.dma_start(out=d_yr.ap(), in_=yr_sb[:])
                    nc.sync.dma_start(out=d_yi.ap(), in_=yi_sb[:])

    nc.compile()
    return nc


def _host_prep(inputs):
    """Build per-core in_maps from the full inputs."""
    import ml_dtypes
    f32 = np.float32
    bf16 = ml_dtypes.bfloat16
    xr = np.ascontiguousarray(inputs["x_real"], dtype=f32).reshape(B, E, HW)
    xi = np.ascontiguousarray(inputs["x_imag"], dtype=f32).reshape(B, E, HW)
    pos = np.asarray(inputs["pos_r"], dtype=f32) + 1j * np.asarray(inputs["pos_i"], dtype=f32)
    w_in_r = np.asarray(inputs["w_in_r"], dtype=f32)
    w_in_i = np.asarray(inputs["w_in_i"], dtype=f32)
    b_in_r = np.asarray(inputs["b_in_r"], dtype=f32)
    b_in_i = np.asarray(inputs["b_in_i"], dtype=f32)
    w_out = np.asarray(inputs["w_out_r"], dtype=f32) + 1j * np.asarray(inputs["w_out_i"], dtype=f32)
    b_out = np.asarray(inputs["b_out_r"], dtype=f32) + 1j * np.asarray(inputs["b_out_i"], dtype=f32)
    w_p = np.asarray(inputs["w_p_r"], dtype=f32) + 1j * np.asarray(inputs["w_p_i"], dtype=f32)
    b_p = np.asarray(inputs["b_p_r"], dtype=f32) + 1j * np.asarray(inputs["b_p_i"], dtype=f32)

    w_in = w_in_r + 1j * w_in_i
    wq, wk, wv = w_in[:E], w_in[E:2 * E], w_in[2 * E:]
    qs = f32(1.0 / np.sqrt(HD))

    posb = np.zeros((E, SP), np.complex64)
    posb[:, :S] = pos

    wc = w_p @ w_out                                        # [OUT, E] complex
    bq = qs * (b_in_r[:E] + 1j * b_in_i[:E])                # [E]

    b_v = b_in_r[2 * E:] + 1j * b_in_i[2 * E:]
    b_c = (1 + 1j) * (b_v @ wc.T) + b_out @ w_p.T + b_p     # [OUT] complex

    mask8 = np.zeros((NH, OUT), f32)
    for h in range(NH):
        mask8[h, h * HD:(h + 1) * HD] = 1.0
    sel32 = np.zeros((32, BPC), f32)
    for b in range(BPC):
        sel32[b * 8:(b + 1) * 8, b] = 1.0

    shared = dict(
        wqr=np.ascontiguousarray(wq.real.T * qs).astype(bf16),
        wqi=np.ascontiguousarray(wq.imag.T * qs).astype(bf16),
        wkr=np.ascontiguousarray(wk.real).astype(bf16),
        wki=np.ascontiguousarray(wk.imag).astype(bf16),
        wvr=np.ascontiguousarray(wv.real.T).astype(bf16),
        wvi=np.ascontiguousarray(wv.imag.T).astype(bf16),
        wcr=np.ascontiguousarray(wc.real.T.astype(f32)).astype(bf16),
        wci=np.ascontiguousarray(wc.imag.T.astype(f32)).astype(bf16),
        bqr=bq.real.astype(f32).reshape(4, 128).T.copy(),
        bqi=bq.imag.astype(f32).reshape(4, 128).T.copy(),
        bcr=np.broadcast_to(b_c.real.astype(f32), (BPC, OUT)).copy(),
        bci=np.broadcast_to(b_c.imag.astype(f32), (BPC, OUT)).copy(),
        ident=np.eye(128, dtype=f32),
        mask8=mask8,
        sel32=sel32.astype(bf16),
        zbd=np.zeros((128, 32), bf16),
    )
    # x_cat fully prepped on host: col 0 = mean, then + pos; col 257 zero
    xrp = np.zeros((B, E, SP), f32)
    xip = np.zeros((B, E, SP), f32)
    xrp[:, :, 1:1 + HW] = xr
    xip[:, :, 1:1 + HW] = xi
    xrp[:, :, 0] = xr.mean(-1)
    xip[:, :, 0] = xi.mean(-1)
    xrp[:, :, :S] += posb.real[None, :, :S]
    xip[:, :, :S] += posb.imag[None, :, :S]
    in_maps = []
    for c in range(NCORES):
        m = dict(shared)
        m["xr"] = np.ascontiguousarray(
            xrp[c * BPC:(c + 1) * BPC].transpose(1, 0, 2)).astype(bf16)
        m["xi"] = np.ascontiguousarray(
            xip[c * BPC:(c + 1) * BPC].transpose(1, 0, 2)).astype(bf16)
        in_maps.append(m)
    return in_maps


def _run(inputs, trace=False, **kw):
    from concourse.bass_utils import run_bass_kernel_spmd
    if "nc" not in _cached:
        _cached["nc"] = _build()
    nc = _cached["nc"]
    in_maps = _host_prep(inputs)
    res = run_bass_kernel_spmd(nc, in_maps, core_ids=list(range(NCORES)),
                               trace=trace, **kw)
    out = np.empty((B, OUT), np.complex64)
    for c in range(NCORES):
        out[c * BPC:(c + 1) * BPC] = (res.results[c]["yr"]
                                      + 1j * res.results[c]["yi"])
    return out, res


def kernel(**inputs) -> np.ndarray:
    out, _ = _run(inputs)
    return out
